# revision 51
# baseline (speedup 1.0000x reference)
"""SwiGLU-projected causal MHA (B=4, S=2048, D=1024, H=16) on 8 TRN2 NeuronCores.

Sharding: core c -> (batch b = c//2, head-group g = c%2).  Each core computes
the SwiGLU Q/K/V projections for its 512 output channels (= 8 heads) of its
batch, runs causal attention for those heads, and produces a partial output
projection (contraction over its 512 channels).  The host sums the two
partials per batch and adds the output bias.

v3: projections and attention are software-pipelined per 512-seq slice:
after projecting q/k/v slice t, attention for query group qg=t runs; its
exp/normalization work (ACT/DVE) overlaps the next slice's projection
matmuls, keeping the PE dense (and its HAM clock at 2.4 GHz).  Projection
pairs and score pairs share one 3-slot PSUM pool so whichever phase is
active gets the double-buffering.

Attention per (qg, pj=head pair): scores S^T [k-part, q-free] for both heads
go to one 2-bank PSUM pair tile; causal masking is PE-side (an identity
matmul accumulates -1e4*triu onto the diagonal 128x128 subtile, and fully
masked column ranges are never computed).  One Exp covers both heads
straight out of PSUM.  For qg>=1 the exp output is written as fp8e4 and the
AV matmuls contract kc pairs with perf_mode=DoubleRow against an fp8 copy
of V (2x PE rate); qg=0 (queries with few visible keys, where quantization
noise does not average out) keeps a bf16 AV path.  V carries a ones column
so the AV emits softmax denominators at output row 64; they are staged at
partitions 0/32 of a persistent tile, inverted with one
reciprocal_approx_fast, and broadcast to 64 rows via K=1 ones matmuls into
the free upper rows of the ctx PSUM banks.
"""
import sys

sys.path.insert(0, "/opt/trn_rl_repo")
import numpy as np

import concourse.bacc as bacc
import concourse.tile as tile
import concourse.mybir as mybir

B, S, D = 4, 2048, 1024
H, DK = 16, 64
NCORES = 8
GCH = 512          # channels per core (8 heads)
NT = S // 128      # 16 seq chunks
F32 = mybir.dt.float32
FP8 = mybir.dt.float8e4
ACTF = mybir.ActivationFunctionType
ALU = mybir.AluOpType
PERF = mybir.MatmulPerfMode
NEG = -10000.0     # additive causal mask (exp(x-1e4) == 0 in fp32)

TRACE = False          # set by test.py for profiling runs
TRACE_CORES = None
LAST_RESULT = None     # BassKernelResults stash for test.py
MM_DTYPE = "bf16"
USE_FP8_AV = True      # fp8 DoubleRow AV for qg>=1 (causal mode only)


def build_program(mask_mode):
    """mask_mode: 'causal' (tril), 'full' (all ones), 'general' (arbitrary)."""
    MMD = mybir.dt.bfloat16 if MM_DTYPE == "bf16" else mybir.dt.float32r
    fp8_av = USE_FP8_AV and mask_mode == "causal"
    nc = bacc.Bacc("TRN2", target_bir_lowering=False, debug=False)

    # q/k projections run as fp8 DoubleRow: x and w pre-packed on the host as
    # [128 part, 4 dc-pair, 2 slot, n] with contraction index 256*m+128*s+p.
    # Weights carry 2^8 (fp8 subnormal avoidance), so qt/kt carry 2^16 and
    # raw scores carry 2^32 -- folded out via the Exp scale argument.
    x8_d = {s: nc.dram_tensor(f"x8{s}", [128, 4, 2, S], FP8,
                              kind="ExternalInput") for s in "qk"}
    w8_d = {f"{wn}{s}": nc.dram_tensor(f"{wn}8_{s}", [128, 4, 2, GCH], FP8,
                                       kind="ExternalInput")
            for s in "qk" for wn in ("w1", "w2")}
    xT = {"v": nc.dram_tensor("xvT", [D, S], MMD, kind="ExternalInput")}
    w1T = {"v": nc.dram_tensor("w1T_v", [D, GCH], MMD, kind="ExternalInput")}
    w2T = {"v": nc.dram_tensor("w2T_v", [D, GCH], MMD, kind="ExternalInput")}
    EXPS = 2.0 ** -32
    bias_d = {}
    for s in "qk":
        for bn in ("b1", "b2", "b1h"):
            bias_d[f"{bn}_{s}"] = nc.dram_tensor(f"{bn}_{s}", [128, 4], F32,
                                                 kind="ExternalInput")
    b1v_d = nc.dram_tensor("b1_v", [1, GCH], MMD, kind="ExternalInput")
    b2v_d = nc.dram_tensor("b2_v", [1, GCH], MMD, kind="ExternalInput")
    woT_d = nc.dram_tensor("woT", [128, 4, D], MMD, kind="ExternalInput")
    mtri_d = m01T_d = None
    if mask_mode == "causal":
        mtri_d = nc.dram_tensor("mtri", [128, 128], MMD, kind="ExternalInput")
    elif mask_mode == "general":
        m01T_d = nc.dram_tensor("m01T", [S, S], MMD, kind="ExternalInput")
    ident_d = nc.dram_tensor("ident", [128, 128], MMD, kind="ExternalInput")
    pout_d = nc.dram_tensor("pout", [S, D], F32, kind="ExternalOutput")

    def kc_count(qg):
        return 4 * qg + 4 if mask_mode == "causal" else NT

    def col0(qg, kc):
        """first valid q column (within the 512 q group) for key block kc."""
        if mask_mode != "causal":
            return 0
        i = kc - 4 * qg
        return 0 if i < 0 else 128 * i

    with tile.TileContext(nc) as tc:
        with (
            tc.tile_pool(name="persist", bufs=1) as persist,
            tc.tile_pool(name="xpool", bufs=10) as xpool,
            tc.tile_pool(name="xpool8", bufs=12) as xpool8,
            tc.tile_pool(name="stage", bufs=3) as stage,
            tc.tile_pool(name="apool", bufs=4) as apool,
            tc.tile_pool(name="apoolb", bufs=2) as apoolb,
            tc.tile_pool(name="smalls", bufs=2) as smalls,
            tc.tile_pool(name="ctpool", bufs=3) as ctpool,
            tc.tile_pool(name="ostage", bufs=2) as ostage,
            tc.tile_pool(name="mpool", bufs=1) as mpool,
            tc.tile_pool(name="work", bufs=3, space="PSUM") as work,
            tc.tile_pool(name="cxps", bufs=2, space="PSUM") as cxps,
        ):
            # ---------------- persistent state ----------------
            qt_t = [persist.tile([128, 4, 512], MMD, tag=f"qt{t}", name=f"qt{t}")
                    for t in range(4)]
            kt_t = [persist.tile([128, 4, 512], MMD, tag=f"kt{t}", name=f"kt{t}")
                    for t in range(4)]
            # bf16 V (per 512-slice) and, in causal mode, an fp8 copy padded
            # to 80 so the DoubleRow kc-pair step stays 16-aligned
            vb_t = [persist.tile([128, 4, 8, 65], MMD, tag=f"vb{t}", name=f"vb{t}")
                    for t in range(4)]
            v8_t = None
            if fp8_av:
                v8_t = [persist.tile([128, 8, 4, 80], FP8, tag=f"v8{t}", name=f"v8{t}")
                        for t in range(4)]
            woT_sb = persist.tile([128, 4, D], MMD, tag="wo")
            onesf = persist.tile([1, 128], F32, tag="onesf")
            ones_r = persist.tile([1, 128], MMD, tag="ones_r")
            onescol = persist.tile([128, 1], F32, tag="onescol")
            ident_sb = persist.tile([128, 128], MMD, tag="ident")
            nc.sync.dma_start(ident_sb[:], ident_d[:])
            if mask_mode == "causal":
                mtri_sb = persist.tile([128, 128], MMD, tag="mtri")
                nc.sync.dma_start(mtri_sb[:], mtri_d[:])
            den_sb = persist.tile([33, 512], F32, tag="den")
            ones33f = persist.tile([33, 128], F32, tag="ones33f")
            ones33 = persist.tile([33, 128], MMD, tag="ones33")

            def deferred_init():
                nc.any.memset(onesf[:], 1.0)
                nc.vector.tensor_copy(ones_r[:], onesf[:])
                nc.any.memset(onescol[:], 1.0)
                for t in range(4):
                    nc.gpsimd.memset(vb_t[t][:, :, :, 64:65], 1.0)
                    if fp8_av:
                        nc.gpsimd.memset(v8_t[t][:, :, :, 64:65], 1.0)
                nc.gpsimd.memset(den_sb[:], 1.0)
                nc.any.memset(ones33f[:], 1.0)
                nc.vector.tensor_copy(ones33[:], ones33f[:])

            # projection weights stay resident; DMA them lazily at first use
            wsb = {}
            bsb = {}

            def load_weights(s):
                if s in "qk":
                    for wn in ("w1", "w2"):
                        wt = persist.tile([128, 4, 2, GCH], FP8,
                                          tag=f"{wn}{s}", name=f"{wn}{s}")
                        nc.gpsimd.dma_start(wt[:], w8_d[f"{wn}{s}"][:])
                        wsb[f"{wn}{s}"] = wt
                else:
                    for wn, wd in (("w1", w1T[s]), ("w2", w2T[s])):
                        wt = persist.tile([128, 8, GCH], MMD, tag=f"{wn}{s}",
                                          name=f"{wn}{s}")
                        for dc in range(8):
                            nc.gpsimd.dma_start(
                                wt[:, dc, :], wd[dc * 128:(dc + 1) * 128, :]
                            )
                        wsb[f"{wn}{s}"] = wt
                if s != "v":
                    for bn in ("b1", "b2", "b1h"):
                        bt = persist.tile([128, 4], F32, tag=f"{bn}{s}",
                                          name=f"{bn}{s}")
                        nc.sync.dma_start(bt[:], bias_d[f"{bn}_{s}"][:])
                        bsb[f"{bn}{s}"] = bt
                else:
                    bsb["b1v"] = b1vr = persist.tile([1, GCH], MMD, tag="b1v",
                                                     name="b1v")
                    bsb["b2v"] = b2vr = persist.tile([1, GCH], MMD, tag="b2v",
                                                     name="b2v")
                    nc.sync.dma_start(b1vr[:], b1v_d[:])
                    nc.sync.dma_start(b2vr[:], b2v_d[:])

            # warm the PE HAM clock while the first weight/x DMAs land; the
            # warmup weights come from a memset (not a DMA) so the first
            # matmul issues as early as possible
            wu_w = persist.tile([128, 128], MMD, tag="wu_w")
            nc.gpsimd.memset(wu_w[:], 0.0)
            # warmup lives in the cxps pool (idle until attention starts) so
            # it never blocks the first projection's work-ring slot
            wu = cxps.tile([128, 512], F32, tag="cx", name="warmup")
            for i in range(96):
                nc.tensor.matmul(
                    wu[:, 0:128], wu_w[:], wu_w[:],
                    start=True, stop=True, skip_group_check=True,
                )

            # ---------------- phase A: one 512-seq slice of s ----------------
            x_pending = {}

            def prefetch_x(s, t):
                if (s, t) in x_pending:
                    return
                xts = []
                if s in "qk":
                    for m in range(4):
                        xt = xpool8.tile([128, 2, 512], FP8, tag="xt8")
                        nc.sync.dma_start(
                            xt[:],
                            x8_d[s][:, m, :, t * 512:(t + 1) * 512],
                        )
                        xts.append(xt)
                else:
                    for dc in range(8):
                        xt = xpool.tile([128, 512], MMD, tag="xt")
                        nc.sync.dma_start(
                            xt[:],
                            xT[s][dc * 128:(dc + 1) * 128,
                                  t * 512:(t + 1) * 512],
                        )
                        xts.append(xt)
                x_pending[(s, t)] = xts

            def project_units(s, t):
                if t == 0:
                    load_weights(s)
                if s == "q" and t == 1:
                    nc.sync.dma_start(woT_sb[:], woT_d[:])
                prefetch_x(s, t)
                xts = x_pending.pop((s, t))
                for j in range(4):
                    pr = work.tile([128, 2, 512], F32, tag="wk")
                    if s == "v":
                        for dc in range(8):
                            # seq on partitions: lhsT = x chunk
                            nc.tensor.matmul(
                                pr[:, 0, :],
                                xts[dc][:, j * 128:(j + 1) * 128],
                                wsb["w1v"][:, dc, :],
                                start=(dc == 0), stop=False,
                            )
                            nc.tensor.matmul(
                                pr[:, 1, :],
                                xts[dc][:, j * 128:(j + 1) * 128],
                                wsb["w2v"][:, dc, :],
                                start=(dc == 0), stop=False,
                            )
                    else:
                        for m in range(4):
                            nc.tensor.matmul(
                                pr[:, 0, :],
                                wsb[f"w1{s}"][:, m, :,
                                              j * 128:(j + 1) * 128],
                                xts[m][:],
                                start=(m == 0), stop=(m == 3),
                                perf_mode=PERF.DoubleRow,
                            )
                            nc.tensor.matmul(
                                pr[:, 1, :],
                                wsb[f"w2{s}"][:, m, :,
                                              j * 128:(j + 1) * 128],
                                xts[m][:],
                                start=(m == 0), stop=(m == 3),
                                perf_mode=PERF.DoubleRow,
                            )
                    act = stage.tile([128, 512], F32, tag="act")
                    if s == "v":
                        # fold the biases into the accumulation (they vary
                        # along the free/channel dim)
                        nc.tensor.matmul(pr[:, 0, :], ones_r[:], bsb["b1v"][:],
                                         start=False, stop=True)
                        nc.tensor.matmul(pr[:, 1, :], ones_r[:], bsb["b2v"][:],
                                         start=False, stop=True)
                        nc.scalar.activation(act[:], pr[:, 0, :],
                                             ACTF.Tanh, scale=0.5)
                        u = stage.tile([128, 512], F32, tag="u")
                        # silu2(A) = (tanh(A/2) + 1) * A in one op
                        nc.vector.scalar_tensor_tensor(
                            u[:], act[:], 1.0, pr[:, 0, :],
                            op0=ALU.add, op1=ALU.mult,
                        )
                        src1 = pr[:, 1, :].rearrange("p (h d) -> p h d", h=8)
                        src2 = u[:].rearrange("p (h d) -> p h d", h=8)
                        nc.vector.tensor_tensor(
                            vb_t[t][:, j, :, 0:64], src1, src2, ALU.mult
                        )
                        if fp8_av:
                            with nc.allow_low_precision(reason="fp8 AV copy"):
                                nc.gpsimd.tensor_copy(
                                    v8_t[t][:, :, j, 0:64],
                                    vb_t[t][:, j, :, 0:64],
                                )
                    else:
                        bias1 = bsb[f"b1{s}"][:, j:j + 1]
                        bias2 = bsb[f"b2{s}"][:, j:j + 1]
                        # q/k PSUM carries 2^8 (fp8 weight pre-scale); fold
                        # the unscale into the tanh affine (b1h is unscaled)
                        nc.scalar.activation(
                            act[:], pr[:, 0, :], ACTF.Tanh,
                            scale=0.5 / 256.0,
                            bias=bsb[f"b1h{s}"][:, j:j + 1],
                        )
                        a_sb = stage.tile([128, 512], F32, tag="u")
                        nc.vector.tensor_scalar_add(a_sb[:], pr[:, 0, :],
                                                    bias1)
                        nc.vector.scalar_tensor_tensor(
                            act[:], act[:], 1.0, a_sb[:],
                            op0=ALU.add, op1=ALU.mult,
                        )
                        dst = (qt_t if s == "q" else kt_t)[t][:, j, :]
                        nc.vector.scalar_tensor_tensor(
                            dst, pr[:, 1, :], bias2, act[:],
                            op0=ALU.add, op1=ALU.mult,
                        )
                    yield

            def project(s, t):
                for _ in project_units(s, t):
                    pass

            # ---------------- phase B: one head pair of one query group -----
            ct_tiles = {}
            ctx_of = {}

            def attention_pair_units(qg, pj, mtiles):
                kcmax = kc_count(qg)
                use8 = fp8_av and qg >= 1
                if pj == 0:
                    ct_tiles[qg] = ctpool.tile([128, 4, 512], MMD, tag="ct",
                                               name=f"ct{qg}")
                ctx = [cxps.tile([128, 512], F32, tag="cx", name=f"ctx_{i}")
                       for i in range(2)]
                ctx_of[(qg, pj)] = ctx
                npair = (kcmax + 1) // 2
                for p in range(npair):
                    attn = (apool if use8 else apoolb).tile(
                        [128, 2, 2, 512], FP8 if use8 else MMD,
                        tag="at8" if use8 else "atb", name=f"at_{qg}_{pj}_{p}",
                    )
                    kcs = [kc for kc in (2 * p, 2 * p + 1) if kc < kcmax]
                    diag_adds = []
                    sc_of = {}
                    for kc in kcs:
                        c0 = col0(qg, kc)
                        tt, lkc = divmod(kc, 4)
                        diag = mask_mode == "causal" and kc >= 4 * qg
                        sc = work.tile([128, 2, 512], F32, tag="wk",
                                           name=f"sc_{qg}_{pj}_{kc}")
                        sc_of[kc] = sc
                        for par in range(2):
                            bp = par * 64
                            nc.tensor.matmul(
                                sc[:, par, c0:],
                                kt_t[tt][bp:bp + 64, pj,
                                         lkc * 128:(lkc + 1) * 128],
                                qt_t[qg][bp:bp + 64, pj, c0:],
                                start=True, stop=not diag,
                            )
                        if diag:
                            diag_adds.append((sc, c0))
                    # batched mask adds: mtri weights stay loaded across all
                    # diag subtiles of the pair (avoids kt<->mtri LDW thrash)
                    for sc, c0 in diag_adds:
                        for par in range(2):
                            nc.tensor.matmul(
                                sc[:, par, c0:c0 + 128],
                                ident_sb[:],
                                mtri_sb[:],
                                start=False, stop=True,
                            )
                    for kc in kcs:
                        c0 = col0(qg, kc)
                        sc = sc_of[kc]
                        with nc.allow_low_precision(reason="attn fp8"):
                            nc.scalar.activation(
                                attn[:, kc & 1, :, c0:], sc[:, :, c0:],
                                ACTF.Exp, scale=EXPS,
                            )
                        if mask_mode == "general":
                            for par in range(2):
                                nc.vector.tensor_tensor(
                                    attn[:, kc & 1, par, :],
                                    attn[:, kc & 1, par, :],
                                    mtiles[kc], ALU.mult,
                                )
                    # ---- AV ----
                    first = (p == 0)
                    last = (p == npair - 1)
                    anydiag = any(mask_mode == "causal" and kc >= 4 * qg
                                  for kc in kcs)
                    for par in range(2):
                        hl = 2 * pj + par
                        if use8 and not anydiag and len(kcs) == 2:
                            tt, l0 = divmod(2 * p, 4)
                            nc.tensor.matmul(
                                ctx[par][0:65, :],
                                v8_t[tt][:, hl, l0:l0 + 2, 0:65],
                                attn[:, :, par, :],
                                start=first, stop=last,
                                perf_mode=PERF.DoubleRow,
                                skip_group_check=True,
                            )
                        else:
                            for kc in kcs:
                                c0 = col0(qg, kc)
                                tt, lkc = divmod(kc, 4)
                                if use8:
                                    vt = v8_t[tt][:, hl, lkc, 0:65]
                                else:
                                    vt = vb_t[tt][:, lkc, hl, :]
                                nc.tensor.matmul(
                                    ctx[par][0:65, c0:],
                                    vt,
                                    attn[:, kc & 1, par, c0:],
                                    start=(first and kc == kcs[0]),
                                    stop=(last and kc == kcs[-1]),
                                    skip_group_check=True,
                                )
                    yield

            def normalize_unit(qg, pj):
                # ---- normalize both heads of the pair into ct_qg ----
                ct_qg = ct_tiles[qg]
                ctx = ctx_of.pop((qg, pj))
                for par in range(2):
                    nc.vector.tensor_copy(
                        den_sb[32 * par:32 * par + 1, :],
                        ctx[par][64:65, :],
                    )
                rec = smalls.tile([33, 512], F32, tag="rec")
                nc.vector.reciprocal_approx_fast(rec[:], den_sb[:])
                rec_b = smalls.tile([33, 512], MMD, tag="recb")
                nc.vector.tensor_copy(rec_b[:], rec[:])
                # broadcast each reciprocal row into the free upper rows
                # (64:128) of its own ctx PSUM bank
                for par in range(2):
                    nc.tensor.matmul(
                        ctx[par][64:128, :],
                        ones33[32 * par:32 * par + 1, 0:64],
                        rec_b[32 * par:32 * par + 1, :],
                        start=True, stop=True,
                        tile_position=(32 * par, 64),
                        skip_group_check=True,
                    )
                bc_sb = smalls.tile([128, 512], F32, tag="bcs")
                for par in range(2):
                    nc.vector.tensor_copy(
                        bc_sb[64 * par:64 * par + 64, :],
                        ctx[par][64:128, :],
                    )
                for par in range(2):
                    bp = par * 64
                    nc.vector.tensor_tensor(
                        ct_qg[bp:bp + 64, pj, :],
                        ctx[par][0:64, :],
                        bc_sb[64 * par:64 * par + 64, :],
                        ALU.mult,
                    )

            def attention_part(qg, pj, mtiles):
                for _ in attention_pair_units(qg, pj, mtiles):
                    pass
                normalize_unit(qg, pj)

            def attention_chain(qg, mtiles):
                for pj in range(4):
                    for _ in attention_pair_units(qg, pj, mtiles):
                        yield
                    normalize_unit(qg, pj)
                    yield

            def proj_chain(t):
                nxt = {"q": ("k", t), "k": ("v", t),
                       "v": ("q", t + 1) if t < 3 else None}
                for s in "qkv":
                    for j, _ in enumerate(project_units(s, t)):
                        if j == 1 and nxt[s] is not None:
                            prefetch_x(*nxt[s])
                        yield

            def weave(main_gen, side_gen, per_unit):
                """emit per_unit side units after each main unit (fractional
                credits), then drain whatever remains."""
                credit = 0.0
                for _ in main_gen:
                    credit += per_unit
                    while credit >= 1.0:
                        if next(side_gen, None) is None:
                            credit = 0.0
                            break
                        credit -= 1.0
                for _ in side_gen:
                    pass

            def attention_out_units(qg):
                ct_qg = ct_tiles[qg]
                for ns in range(4):
                    nt_i = qg * 4 + ns
                    nsl = slice(ns * 128, (ns + 1) * 128)
                    po = work.tile([128, 2, 512], F32, tag="wk",
                                   name=f"po_{qg}_{ns}")
                    for oh in range(2):
                        for j in range(4):
                            nc.tensor.matmul(
                                po[:, oh, :],
                                ct_qg[:, j, nsl],
                                woT_sb[:, j, oh * 512:(oh + 1) * 512],
                                start=(j == 0), stop=(j == 3),
                            )
                    ot = ostage.tile([128, 1024], F32, tag="ot")
                    nc.vector.tensor_copy(ot[:], po[:])
                    nc.sync.dma_start(
                        pout_d[nt_i * 128:(nt_i + 1) * 128, :],
                        ot[:],
                    )
                    yield

            def attention_out(qg):
                for _ in attention_out_units(qg):
                    pass

            def general_mtiles(qg):
                if mask_mode != "general":
                    return None
                kcmax = kc_count(qg)
                mtiles = []
                mt_sb = mpool.tile([128, NT, 512], MMD, tag="mt")
                for kc in range(kcmax):
                    nc.sync.dma_start(
                        mt_sb[:, kc, :],
                        m01T_d[kc * 128:(kc + 1) * 128,
                               qg * 512:(qg + 1) * 512],
                    )
                    mtiles.append(mt_sb[:, kc, :])
                return mtiles

            # ---------------- interleaved schedule ----------------
            # Unit-level weave: attention(qg=t-1)'s kc-pair units are the main
            # stream; the three projection streams of slice t (plus, from t=2,
            # the output projection of group t-2) are the side stream.  The
            # fine interleave keeps proj matmuls between attention pairs so
            # the PE never outruns ACT's exp stream (which is slower per kc
            # pair than the PE work it gates), and the HAM clock stays warm.
            deferred_init()
            if mask_mode == "causal":
                import itertools

                for _ in proj_chain(0):
                    pass
                for t in range(1, 4):
                    qg = t - 1
                    main = attention_chain(qg, None)
                    side = proj_chain(t)
                    n_main = 4 * (((kc_count(qg) + 1) // 2) + 1)
                    n_side = 12
                    if qg == 2:
                        side = itertools.chain(side, attention_out_units(0))
                        n_side += 4
                    weave(main, side, n_side / n_main)
                main = attention_chain(3, None)
                side = itertools.chain(attention_out_units(1),
                                       attention_out_units(2))
                weave(main, side, 8 / 36)
                attention_out(3)
            else:
                for t in range(4):
                    for s in "qkv":
                        project(s, t)
                for qg in range(4):
                    mtiles = general_mtiles(qg)
                    for pj in range(4):
                        attention_part(qg, pj, mtiles)
                    attention_out(qg)
    nc.compile()
    return nc


def _host_prepare(inputs):
    """Split the full problem into 8 per-core input maps + host-side info."""
    q = np.asarray(inputs["query"], dtype=np.float32)
    k = np.asarray(inputs["key"], dtype=np.float32)
    v = np.asarray(inputs["value"], dtype=np.float32)
    mask = np.asarray(inputs["mask"])
    w = {n: np.asarray(inputs[n], dtype=np.float32)
         for n in ("wq1", "wq2", "wk1", "wk2", "wv1", "wv2", "wo")}
    bias = {n: np.asarray(inputs[n], dtype=np.float32)
            for n in ("bq1", "bq2", "bk1", "bk2", "bv1", "bv2", "bo")}

    m = mask.reshape(S, S)
    if np.array_equal(m != 0, np.tril(np.ones((S, S), bool))):
        mask_mode = "causal"
    elif np.all(m != 0):
        mask_mode = "full"
    else:
        mask_mode = "general"

    m01T = None
    if mask_mode == "general":
        m01T = np.ascontiguousarray((m != 0).T.astype(np.float32))

    scale = 1.0 / np.sqrt(DK).astype(np.float32)

    if MM_DTYPE == "bf16":
        import ml_dtypes

        mmd_np = ml_dtypes.bfloat16
    else:
        mmd_np = np.float32

    def cvt(a):
        return np.ascontiguousarray(a).astype(mmd_np)

    kk = np.arange(128)[:, None]
    qq = np.arange(128)[None, :]
    # scores carry 2^32 (each of qt/kt carries 2^16); the mask add must be in
    # the same scaled units (folded back out by the Exp scale argument)
    mtri = (kk > qq).astype(np.float32) * (NEG * 2.0 ** 32)
    ident = np.eye(128, dtype=np.float32)

    import ml_dtypes as mld

    def pack8(a):
        """[D, n] -> fp8 DoubleRow layout [128, 4, 2, n]."""
        a = np.asarray(a, np.float32).reshape(4, 2, 128, -1)
        a = np.clip(a, -240.0, 240.0).transpose(2, 0, 1, 3)
        return np.ascontiguousarray(a).astype(mld.float8_e4m3)

    WS = 256.0   # fp8 weight pre-scale (2^8)

    in_maps = []
    for c in range(NCORES):
        b, g = divmod(c, 2)
        sl = slice(g * GCH, (g + 1) * GCH)
        im = {
            "x8q": pack8(q[b].T),
            "x8k": pack8(k[b].T),
            "xvT": cvt(v[b].T),
            "w18_q": pack8(w["wq1"][sl].T * WS),
            # fold the 1/sqrt(dk) score scale into the non-silu Q branch,
            # and 0.5 everywhere (silu computed as A*(1+tanh(A/2)) = 2*silu)
            "w28_q": pack8(w["wq2"][sl].T * (scale * 0.5 * WS)),
            "w18_k": pack8(w["wk1"][sl].T * WS),
            "w28_k": pack8(w["wk2"][sl].T * (0.5 * WS)),
            "w1T_v": cvt(w["wv1"][sl].T),
            "w2T_v": cvt(w["wv2"][sl].T * 0.5),
            "b1_q": np.ascontiguousarray(
                (bias["bq1"][sl] * WS).reshape(4, 128).T),
            "b1h_q": np.ascontiguousarray(
                (bias["bq1"][sl] * 0.5).reshape(4, 128).T),
            "b2_q": np.ascontiguousarray(
                (bias["bq2"][sl] * (scale * 0.5 * WS)).reshape(4, 128).T),
            "b1_k": np.ascontiguousarray(
                (bias["bk1"][sl] * WS).reshape(4, 128).T),
            "b1h_k": np.ascontiguousarray(
                (bias["bk1"][sl] * 0.5).reshape(4, 128).T),
            "b2_k": np.ascontiguousarray(
                (bias["bk2"][sl] * (0.5 * WS)).reshape(4, 128).T),
            "b1_v": cvt(bias["bv1"][sl].reshape(1, GCH)),
            "b2_v": cvt((bias["bv2"][sl] * 0.5).reshape(1, GCH)),
            "woT": cvt(
                w["wo"][:, sl].T.reshape(4, 128, D).transpose(1, 0, 2)),
            "ident": cvt(ident),
        }
        if mask_mode == "causal":
            im["mtri"] = cvt(mtri)
        elif mask_mode == "general":
            im["m01T"] = cvt(m01T)
        in_maps.append(im)
    return mask_mode, in_maps, bias["bo"]


LAST_NC = None


def kernel(**inputs):
    global LAST_RESULT, LAST_NC
    mask_mode, in_maps, bo = _host_prepare(inputs)
    nc = build_program(mask_mode)
    LAST_NC = nc

    import concourse.bass_utils as bu

    if TRACE:
        import types

        try:
            from trn_agent_boot.trn_boot import _ntff_profile_via_ctypes

            hook = _ntff_profile_via_ctypes("/opt/axon/libaxon_pjrt.so")
            m = types.ModuleType("antenv.axon_hooks")
            m.get_axon_ntff_profile_hook = lambda: hook
            import antenv  # noqa: F401

            sys.modules["antenv.axon_hooks"] = m
            bu.upload_artifacts = lambda d: "local://skipped"
        except Exception as e:
            print("profiling hook install failed:", e)

    res = bu.run_bass_kernel_spmd(
        nc, in_maps, core_ids=list(range(NCORES)),
        trace=TRACE, trace_cores=TRACE_CORES,
    )
    LAST_RESULT = res

    out = np.empty((B, S, D), dtype=np.float32)
    for b in range(B):
        out[b] = (res.results[2 * b]["pout"] + res.results[2 * b + 1]["pout"]
                  + bo[None, :])
    return out



# revision 54
# speedup vs baseline: 1.1611x; 1.1611x over previous
"""SwiGLU-projected causal MHA (B=4, S=2048, D=1024, H=16) on 8 TRN2 NeuronCores.

Sharding: core c -> (batch b = c//2, head-group g = c%2).  Each core computes
the SwiGLU Q/K/V projections for its 512 output channels (= 8 heads) of its
batch, runs causal attention for those heads, and produces a partial output
projection (contraction over its 512 channels).  The host sums the two
partials per batch and adds the output bias.

v3: projections and attention are software-pipelined per 512-seq slice:
after projecting q/k/v slice t, attention for query group qg=t runs; its
exp/normalization work (ACT/DVE) overlaps the next slice's projection
matmuls, keeping the PE dense (and its HAM clock at 2.4 GHz).  Projection
pairs and score pairs share one 3-slot PSUM pool so whichever phase is
active gets the double-buffering.

Attention per (qg, pj=head pair): scores S^T [k-part, q-free] for both heads
go to one 2-bank PSUM pair tile; causal masking is PE-side (an identity
matmul accumulates -1e4*triu onto the diagonal 128x128 subtile, and fully
masked column ranges are never computed).  One Exp covers both heads
straight out of PSUM.  For qg>=1 the exp output is written as fp8e4 and the
AV matmuls contract kc pairs with perf_mode=DoubleRow against an fp8 copy
of V (2x PE rate); qg=0 (queries with few visible keys, where quantization
noise does not average out) keeps a bf16 AV path.  V carries a ones column
so the AV emits softmax denominators at output row 64; they are staged at
partitions 0/32 of a persistent tile, inverted with one
reciprocal_approx_fast, and broadcast to 64 rows via K=1 ones matmuls into
the free upper rows of the ctx PSUM banks.
"""
import sys

sys.path.insert(0, "/opt/trn_rl_repo")
import numpy as np

import concourse.bacc as bacc
import concourse.tile as tile
import concourse.mybir as mybir

B, S, D = 4, 2048, 1024
H, DK = 16, 64
NCORES = 8
GCH = 512          # channels per core (8 heads)
NT = S // 128      # 16 seq chunks
F32 = mybir.dt.float32
FP8 = mybir.dt.float8e4
ACTF = mybir.ActivationFunctionType
ALU = mybir.AluOpType
PERF = mybir.MatmulPerfMode
NEG = -10000.0     # additive causal mask (exp(x-1e4) == 0 in fp32)

TRACE = False          # set by test.py for profiling runs
TRACE_CORES = None
LAST_RESULT = None     # BassKernelResults stash for test.py
MM_DTYPE = "bf16"
USE_FP8_AV = True      # fp8 DoubleRow AV for qg>=1 (causal mode only)


def build_program(mask_mode):
    """mask_mode: 'causal' (tril), 'full' (all ones), 'general' (arbitrary)."""
    MMD = mybir.dt.bfloat16 if MM_DTYPE == "bf16" else mybir.dt.float32r
    fp8_av = USE_FP8_AV and mask_mode == "causal"
    nc = bacc.Bacc("TRN2", target_bir_lowering=False, debug=False)

    # q/k projections run as fp8 DoubleRow: x and w pre-packed on the host as
    # [128 part, 4 dc-pair, 2 slot, n] with contraction index 256*m+128*s+p.
    # Weights carry 2^8 (fp8 subnormal avoidance), so qt/kt carry 2^16 and
    # raw scores carry 2^32 -- folded out via the Exp scale argument.
    x8_d = {s: nc.dram_tensor(f"x8{s}", [128, 4, 2, S], FP8,
                              kind="ExternalInput") for s in "qk"}
    w8_d = {f"{wn}{s}": nc.dram_tensor(f"{wn}8_{s}", [128, 4, 2, GCH], FP8,
                                       kind="ExternalInput")
            for s in "qk" for wn in ("w1", "w2")}
    xT = {"v": nc.dram_tensor("xvT", [D, S], MMD, kind="ExternalInput")}
    w1T = {"v": nc.dram_tensor("w1T_v", [D, GCH], MMD, kind="ExternalInput")}
    w2T = {"v": nc.dram_tensor("w2T_v", [D, GCH], MMD, kind="ExternalInput")}
    EXPS = 2.0 ** -32
    bias_d = {}
    for s in "qk":
        for bn in ("b1", "b2", "b1h"):
            bias_d[f"{bn}_{s}"] = nc.dram_tensor(f"{bn}_{s}", [128, 4], F32,
                                                 kind="ExternalInput")
    b1v_d = nc.dram_tensor("b1_v", [1, GCH], MMD, kind="ExternalInput")
    b2v_d = nc.dram_tensor("b2_v", [1, GCH], MMD, kind="ExternalInput")
    woT_d = nc.dram_tensor("woT", [128, 4, D], MMD, kind="ExternalInput")
    mtri_d = m01T_d = None
    if mask_mode == "causal":
        mtri_d = nc.dram_tensor("mtri", [128, 128], MMD, kind="ExternalInput")
    elif mask_mode == "general":
        m01T_d = nc.dram_tensor("m01T", [S, S], MMD, kind="ExternalInput")
    ident_d = nc.dram_tensor("ident", [128, 128], MMD, kind="ExternalInput")
    pout_d = nc.dram_tensor("pout", [S, D], F32, kind="ExternalOutput")

    def kc_count(qg):
        return 4 * qg + 4 if mask_mode == "causal" else NT

    def col0(qg, kc):
        """first valid q column (within the 512 q group) for key block kc."""
        if mask_mode != "causal":
            return 0
        i = kc - 4 * qg
        return 0 if i < 0 else 128 * i

    with tile.TileContext(nc) as tc:
        with (
            tc.tile_pool(name="persist", bufs=1) as persist,
            tc.tile_pool(name="xpool", bufs=10) as xpool,
            tc.tile_pool(name="xpool8", bufs=12) as xpool8,
            tc.tile_pool(name="stage", bufs=3) as stage,
            tc.tile_pool(name="apool", bufs=4) as apool,
            tc.tile_pool(name="apoolb", bufs=2) as apoolb,
            tc.tile_pool(name="smalls", bufs=2) as smalls,
            tc.tile_pool(name="ctpool", bufs=3) as ctpool,
            tc.tile_pool(name="ostage", bufs=2) as ostage,
            tc.tile_pool(name="mpool", bufs=1) as mpool,
            tc.tile_pool(name="work", bufs=3, space="PSUM") as work,
            tc.tile_pool(name="cxps", bufs=2, space="PSUM") as cxps,
        ):
            # ---------------- persistent state ----------------
            qt_t = [persist.tile([128, 4, 512], MMD, tag=f"qt{t}", name=f"qt{t}")
                    for t in range(4)]
            kt_t = [persist.tile([128, 4, 512], MMD, tag=f"kt{t}", name=f"kt{t}")
                    for t in range(4)]
            # bf16 V (per 512-slice) and, in causal mode, an fp8 copy padded
            # to 80 so the DoubleRow kc-pair step stays 16-aligned
            vb_t = [persist.tile([128, 4, 8, 65], MMD, tag=f"vb{t}", name=f"vb{t}")
                    for t in range(4)]
            v8_t = None
            if fp8_av:
                v8_t = [persist.tile([128, 8, 4, 80], FP8, tag=f"v8{t}", name=f"v8{t}")
                        for t in range(4)]
            woT_sb = persist.tile([128, 4, D], MMD, tag="wo")
            onesf = persist.tile([1, 128], F32, tag="onesf")
            ones_r = persist.tile([1, 128], MMD, tag="ones_r")
            onescol = persist.tile([128, 1], F32, tag="onescol")
            ident_sb = persist.tile([128, 128], MMD, tag="ident")
            nc.sync.dma_start(ident_sb[:], ident_d[:])
            if mask_mode == "causal":
                mtri_sb = persist.tile([128, 128], MMD, tag="mtri")
                nc.sync.dma_start(mtri_sb[:], mtri_d[:])
            den_sb = persist.tile([33, 512], F32, tag="den")
            ones33f = persist.tile([33, 128], F32, tag="ones33f")
            ones33 = persist.tile([33, 128], MMD, tag="ones33")

            def deferred_init():
                nc.any.memset(onesf[:], 1.0)
                nc.vector.tensor_copy(ones_r[:], onesf[:])
                nc.any.memset(onescol[:], 1.0)
                for t in range(4):
                    nc.gpsimd.memset(vb_t[t][:, :, :, 64:65], 1.0)
                    if fp8_av:
                        nc.gpsimd.memset(v8_t[t][:, :, :, 64:65], 1.0)
                nc.gpsimd.memset(den_sb[:], 1.0)
                nc.any.memset(ones33f[:], 1.0)
                nc.vector.tensor_copy(ones33[:], ones33f[:])

            # projection weights stay resident; DMA them lazily at first use
            wsb = {}
            bsb = {}

            def load_weights(s):
                if s in "qk":
                    for wn in ("w1", "w2"):
                        wt = persist.tile([128, 4, 2, GCH], FP8,
                                          tag=f"{wn}{s}", name=f"{wn}{s}")
                        nc.gpsimd.dma_start(wt[:], w8_d[f"{wn}{s}"][:])
                        wsb[f"{wn}{s}"] = wt
                else:
                    for wn, wd in (("w1", w1T[s]), ("w2", w2T[s])):
                        wt = persist.tile([128, 8, GCH], MMD, tag=f"{wn}{s}",
                                          name=f"{wn}{s}")
                        for dc in range(8):
                            nc.gpsimd.dma_start(
                                wt[:, dc, :], wd[dc * 128:(dc + 1) * 128, :]
                            )
                        wsb[f"{wn}{s}"] = wt
                if s != "v":
                    for bn in ("b1", "b2", "b1h"):
                        bt = persist.tile([128, 4], F32, tag=f"{bn}{s}",
                                          name=f"{bn}{s}")
                        nc.sync.dma_start(bt[:], bias_d[f"{bn}_{s}"][:])
                        bsb[f"{bn}{s}"] = bt
                else:
                    bsb["b1v"] = b1vr = persist.tile([1, GCH], MMD, tag="b1v",
                                                     name="b1v")
                    bsb["b2v"] = b2vr = persist.tile([1, GCH], MMD, tag="b2v",
                                                     name="b2v")
                    nc.sync.dma_start(b1vr[:], b1v_d[:])
                    nc.sync.dma_start(b2vr[:], b2v_d[:])

            # warm the PE HAM clock while the first weight/x DMAs land; the
            # warmup weights come from a memset (not a DMA) so the first
            # matmul issues as early as possible
            wu_w = persist.tile([128, 128], MMD, tag="wu_w")
            nc.gpsimd.memset(wu_w[:], 0.0)
            # warmup lives in the cxps pool (idle until attention starts) so
            # it never blocks the first projection's work-ring slot
            wu = cxps.tile([128, 512], F32, tag="cx", name="warmup")
            for i in range(96):
                nc.tensor.matmul(
                    wu[:, 0:128], wu_w[:], wu_w[:],
                    start=True, stop=True, skip_group_check=True,
                )

            # ---------------- phase A: one 512-seq slice of s ----------------
            x_pending = {}

            def prefetch_x(s, t):
                if (s, t) in x_pending:
                    return
                xts = []
                if s in "qk":
                    for m in range(4):
                        xt = xpool8.tile([128, 2, 512], FP8, tag="xt8")
                        nc.sync.dma_start(
                            xt[:],
                            x8_d[s][:, m, :, t * 512:(t + 1) * 512],
                        )
                        xts.append(xt)
                else:
                    for dc in range(8):
                        xt = xpool.tile([128, 512], MMD, tag="xt")
                        nc.sync.dma_start(
                            xt[:],
                            xT[s][dc * 128:(dc + 1) * 128,
                                  t * 512:(t + 1) * 512],
                        )
                        xts.append(xt)
                x_pending[(s, t)] = xts

            def project_units(s, t):
                if t == 0:
                    load_weights(s)
                if s == "q" and t == 1:
                    nc.sync.dma_start(woT_sb[:], woT_d[:])
                prefetch_x(s, t)
                xts = x_pending.pop((s, t))
                for j in range(4):
                    pr = work.tile([128, 2, 512], F32, tag="wk")
                    if s == "v":
                        for dc in range(8):
                            # seq on partitions: lhsT = x chunk
                            nc.tensor.matmul(
                                pr[:, 0, :],
                                xts[dc][:, j * 128:(j + 1) * 128],
                                wsb["w1v"][:, dc, :],
                                start=(dc == 0), stop=False,
                            )
                            nc.tensor.matmul(
                                pr[:, 1, :],
                                xts[dc][:, j * 128:(j + 1) * 128],
                                wsb["w2v"][:, dc, :],
                                start=(dc == 0), stop=False,
                            )
                    else:
                        for m in range(4):
                            nc.tensor.matmul(
                                pr[:, 0, :],
                                wsb[f"w1{s}"][:, m, :,
                                              j * 128:(j + 1) * 128],
                                xts[m][:],
                                start=(m == 0), stop=(m == 3),
                                perf_mode=PERF.DoubleRow,
                            )
                            nc.tensor.matmul(
                                pr[:, 1, :],
                                wsb[f"w2{s}"][:, m, :,
                                              j * 128:(j + 1) * 128],
                                xts[m][:],
                                start=(m == 0), stop=(m == 3),
                                perf_mode=PERF.DoubleRow,
                            )
                    act = stage.tile([128, 512], F32, tag="act")
                    if s == "v":
                        # fold the biases into the accumulation (they vary
                        # along the free/channel dim)
                        nc.tensor.matmul(pr[:, 0, :], ones_r[:], bsb["b1v"][:],
                                         start=False, stop=True)
                        nc.tensor.matmul(pr[:, 1, :], ones_r[:], bsb["b2v"][:],
                                         start=False, stop=True)
                        nc.scalar.activation(act[:], pr[:, 0, :],
                                             ACTF.Tanh, scale=0.5)
                        u = stage.tile([128, 512], F32, tag="u")
                        # silu2(A) = (tanh(A/2) + 1) * A in one op
                        nc.vector.scalar_tensor_tensor(
                            u[:], act[:], 1.0, pr[:, 0, :],
                            op0=ALU.add, op1=ALU.mult,
                        )
                        src1 = pr[:, 1, :].rearrange("p (h d) -> p h d", h=8)
                        src2 = u[:].rearrange("p (h d) -> p h d", h=8)
                        nc.vector.tensor_tensor(
                            vb_t[t][:, j, :, 0:64], src1, src2, ALU.mult
                        )
                        if fp8_av:
                            with nc.allow_low_precision(reason="fp8 AV copy"):
                                nc.gpsimd.tensor_copy(
                                    v8_t[t][:, :, j, 0:64],
                                    vb_t[t][:, j, :, 0:64],
                                )
                    else:
                        bias1 = bsb[f"b1{s}"][:, j:j + 1]
                        bias2 = bsb[f"b2{s}"][:, j:j + 1]
                        # q/k PSUM carries 2^8 (fp8 weight pre-scale); fold
                        # the unscale into the tanh affine (b1h is unscaled)
                        nc.scalar.activation(
                            act[:], pr[:, 0, :], ACTF.Tanh,
                            scale=0.5 / 256.0,
                            bias=bsb[f"b1h{s}"][:, j:j + 1],
                        )
                        a_sb = stage.tile([128, 512], F32, tag="u")
                        nc.vector.tensor_scalar_add(a_sb[:], pr[:, 0, :],
                                                    bias1)
                        nc.vector.scalar_tensor_tensor(
                            act[:], act[:], 1.0, a_sb[:],
                            op0=ALU.add, op1=ALU.mult,
                        )
                        dst = (qt_t if s == "q" else kt_t)[t][:, j, :]
                        nc.vector.scalar_tensor_tensor(
                            dst, pr[:, 1, :], bias2, act[:],
                            op0=ALU.add, op1=ALU.mult,
                        )
                    yield

            def project(s, t):
                for _ in project_units(s, t):
                    pass

            # ---------------- phase B: one head pair of one query group -----
            ct_tiles = {}
            ctx_of = {}

            def attention_pair_units(qg, pj, mtiles):
                kcmax = kc_count(qg)
                use8 = fp8_av and qg >= 1
                if pj == 0:
                    ct_tiles[qg] = ctpool.tile([128, 4, 512], MMD, tag="ct",
                                               name=f"ct{qg}")
                ctx = [cxps.tile([128, 512], F32, tag="cx", name=f"ctx_{i}")
                       for i in range(2)]
                ctx_of[(qg, pj)] = ctx
                npair = (kcmax + 1) // 2
                for p in range(npair):
                    attn = (apool if use8 else apoolb).tile(
                        [128, 2, 2, 512], FP8 if use8 else MMD,
                        tag="at8" if use8 else "atb", name=f"at_{qg}_{pj}_{p}",
                    )
                    kcs = [kc for kc in (2 * p, 2 * p + 1) if kc < kcmax]
                    diag_adds = []
                    sc_of = {}
                    for kc in kcs:
                        c0 = col0(qg, kc)
                        tt, lkc = divmod(kc, 4)
                        diag = mask_mode == "causal" and kc >= 4 * qg
                        sc = work.tile([128, 2, 512], F32, tag="wk",
                                           name=f"sc_{qg}_{pj}_{kc}")
                        sc_of[kc] = sc
                        for par in range(2):
                            bp = par * 64
                            nc.tensor.matmul(
                                sc[:, par, c0:],
                                kt_t[tt][bp:bp + 64, pj,
                                         lkc * 128:(lkc + 1) * 128],
                                qt_t[qg][bp:bp + 64, pj, c0:],
                                start=True, stop=not diag,
                            )
                        if diag:
                            diag_adds.append((sc, c0))
                    # batched mask adds: mtri weights stay loaded across all
                    # diag subtiles of the pair (avoids kt<->mtri LDW thrash)
                    for sc, c0 in diag_adds:
                        for par in range(2):
                            nc.tensor.matmul(
                                sc[:, par, c0:c0 + 128],
                                ident_sb[:],
                                mtri_sb[:],
                                start=False, stop=True,
                            )
                    for kc in kcs:
                        c0 = col0(qg, kc)
                        sc = sc_of[kc]
                        with nc.allow_low_precision(reason="attn fp8"):
                            nc.scalar.activation(
                                attn[:, kc & 1, :, c0:], sc[:, :, c0:],
                                ACTF.Exp, scale=EXPS,
                            )
                        if mask_mode == "general":
                            for par in range(2):
                                nc.vector.tensor_tensor(
                                    attn[:, kc & 1, par, :],
                                    attn[:, kc & 1, par, :],
                                    mtiles[kc], ALU.mult,
                                )
                    # ---- AV ----
                    first = (p == 0)
                    last = (p == npair - 1)
                    anydiag = any(mask_mode == "causal" and kc >= 4 * qg
                                  for kc in kcs)
                    for par in range(2):
                        hl = 2 * pj + par
                        if use8 and not anydiag and len(kcs) == 2:
                            tt, l0 = divmod(2 * p, 4)
                            nc.tensor.matmul(
                                ctx[par][0:65, :],
                                v8_t[tt][:, hl, l0:l0 + 2, 0:65],
                                attn[:, :, par, :],
                                start=first, stop=last,
                                perf_mode=PERF.DoubleRow,
                                skip_group_check=True,
                            )
                        else:
                            for kc in kcs:
                                c0 = col0(qg, kc)
                                tt, lkc = divmod(kc, 4)
                                if use8:
                                    vt = v8_t[tt][:, hl, lkc, 0:65]
                                else:
                                    vt = vb_t[tt][:, lkc, hl, :]
                                nc.tensor.matmul(
                                    ctx[par][0:65, c0:],
                                    vt,
                                    attn[:, kc & 1, par, c0:],
                                    start=(first and kc == kcs[0]),
                                    stop=(last and kc == kcs[-1]),
                                    skip_group_check=True,
                                )
                    yield

            def normalize_unit(qg, pj):
                # ---- normalize both heads of the pair into ct_qg ----
                ct_qg = ct_tiles[qg]
                ctx = ctx_of.pop((qg, pj))
                for par in range(2):
                    nc.vector.tensor_copy(
                        den_sb[32 * par:32 * par + 1, :],
                        ctx[par][64:65, :],
                    )
                rec = smalls.tile([33, 512], F32, tag="rec")
                nc.vector.reciprocal_approx_fast(rec[:], den_sb[:])
                rec_b = smalls.tile([33, 512], MMD, tag="recb")
                nc.vector.tensor_copy(rec_b[:], rec[:])
                # broadcast each reciprocal row into the free upper rows
                # (64:128) of its own ctx PSUM bank
                for par in range(2):
                    nc.tensor.matmul(
                        ctx[par][64:128, :],
                        ones33[32 * par:32 * par + 1, 0:64],
                        rec_b[32 * par:32 * par + 1, :],
                        start=True, stop=True,
                        tile_position=(32 * par, 64),
                        skip_group_check=True,
                    )
                bc_sb = smalls.tile([128, 512], F32, tag="bcs")
                for par in range(2):
                    nc.vector.tensor_copy(
                        bc_sb[64 * par:64 * par + 64, :],
                        ctx[par][64:128, :],
                    )
                for par in range(2):
                    bp = par * 64
                    nc.vector.tensor_tensor(
                        ct_qg[bp:bp + 64, pj, :],
                        ctx[par][0:64, :],
                        bc_sb[64 * par:64 * par + 64, :],
                        ALU.mult,
                    )

            def attention_part(qg, pj, mtiles):
                for _ in attention_pair_units(qg, pj, mtiles):
                    pass
                normalize_unit(qg, pj)

            def attention_chain(qg, mtiles):
                for pj in range(4):
                    for _ in attention_pair_units(qg, pj, mtiles):
                        yield
                    normalize_unit(qg, pj)
                    yield

            def proj_stream(s, t, prefetches=()):
                for j, _ in enumerate(project_units(s, t)):
                    if j == 1:
                        for nx in prefetches:
                            prefetch_x(*nx)
                    yield

            def proj_chain(t):
                nxt = {"q": [("k", t)], "k": [("v", t)],
                       "v": [("q", t + 1)] if t < 3 else []}
                for s in "qkv":
                    yield from proj_stream(s, t, nxt[s])

            def weave(main_gen, side_gen, per_unit):
                """emit per_unit side units after each main unit (fractional
                credits), then drain whatever remains."""
                credit = 0.0
                for _ in main_gen:
                    credit += per_unit
                    while credit >= 1.0:
                        if next(side_gen, None) is None:
                            credit = 0.0
                            break
                        credit -= 1.0
                for _ in side_gen:
                    pass

            def attention_out_units(qg):
                ct_qg = ct_tiles[qg]
                for ns in range(4):
                    nt_i = qg * 4 + ns
                    nsl = slice(ns * 128, (ns + 1) * 128)
                    po = work.tile([128, 2, 512], F32, tag="wk",
                                   name=f"po_{qg}_{ns}")
                    for oh in range(2):
                        for j in range(4):
                            nc.tensor.matmul(
                                po[:, oh, :],
                                ct_qg[:, j, nsl],
                                woT_sb[:, j, oh * 512:(oh + 1) * 512],
                                start=(j == 0), stop=(j == 3),
                            )
                    ot = ostage.tile([128, 1024], F32, tag="ot")
                    nc.vector.tensor_copy(ot[:], po[:])
                    nc.sync.dma_start(
                        pout_d[nt_i * 128:(nt_i + 1) * 128, :],
                        ot[:],
                    )
                    yield

            def attention_out(qg):
                for _ in attention_out_units(qg):
                    pass

            def general_mtiles(qg):
                if mask_mode != "general":
                    return None
                kcmax = kc_count(qg)
                mtiles = []
                mt_sb = mpool.tile([128, NT, 512], MMD, tag="mt")
                for kc in range(kcmax):
                    nc.sync.dma_start(
                        mt_sb[:, kc, :],
                        m01T_d[kc * 128:(kc + 1) * 128,
                               qg * 512:(qg + 1) * 512],
                    )
                    mtiles.append(mt_sb[:, kc, :])
                return mtiles

            # ---------------- interleaved schedule ----------------
            # Unit-level weave: attention(qg=t-1)'s kc-pair units are the main
            # stream; the three projection streams of slice t (plus, from t=2,
            # the output projection of group t-2) are the side stream.  The
            # fine interleave keeps proj matmuls between attention pairs so
            # the PE never outruns ACT's exp stream (which is slower per kc
            # pair than the PE work it gates), and the HAM clock stays warm.
            deferred_init()
            if mask_mode == "causal":
                import itertools

                for _ in proj_chain(0):
                    pass
                for t in range(1, 3):
                    qg = t - 1
                    main = attention_chain(qg, None)
                    side = proj_chain(t)
                    n_main = 4 * (((kc_count(qg) + 1) // 2) + 1)
                    weave(main, side, 12 / n_main)
                # round t=3: q and v projections of slice 3; the k(3) stream
                # moves into the exp-bound final round, where the PE otherwise
                # starves (attention(3) needs kt3[:, pj, :] only for its last
                # key blocks of head-pair pj, which the weave reaches in time)
                main = attention_chain(2, None)
                side = itertools.chain(
                    proj_stream("q", 3, [("k", 3), ("v", 3)]),
                    proj_stream("v", 3),
                    attention_out_units(0),
                )
                weave(main, side, 12 / 28)
                main = attention_chain(3, None)
                side = itertools.chain(
                    proj_stream("k", 3),
                    attention_out_units(1), attention_out_units(2),
                )
                weave(main, side, 12 / 36)
                attention_out(3)
            else:
                for t in range(4):
                    for s in "qkv":
                        project(s, t)
                for qg in range(4):
                    mtiles = general_mtiles(qg)
                    for pj in range(4):
                        attention_part(qg, pj, mtiles)
                    attention_out(qg)
    nc.compile()
    return nc


def _host_prepare(inputs):
    """Split the full problem into 8 per-core input maps + host-side info."""
    q = np.asarray(inputs["query"], dtype=np.float32)
    k = np.asarray(inputs["key"], dtype=np.float32)
    v = np.asarray(inputs["value"], dtype=np.float32)
    mask = np.asarray(inputs["mask"])
    w = {n: np.asarray(inputs[n], dtype=np.float32)
         for n in ("wq1", "wq2", "wk1", "wk2", "wv1", "wv2", "wo")}
    bias = {n: np.asarray(inputs[n], dtype=np.float32)
            for n in ("bq1", "bq2", "bk1", "bk2", "bv1", "bv2", "bo")}

    m = mask.reshape(S, S)
    if np.array_equal(m != 0, np.tril(np.ones((S, S), bool))):
        mask_mode = "causal"
    elif np.all(m != 0):
        mask_mode = "full"
    else:
        mask_mode = "general"

    m01T = None
    if mask_mode == "general":
        m01T = np.ascontiguousarray((m != 0).T.astype(np.float32))

    scale = 1.0 / np.sqrt(DK).astype(np.float32)

    if MM_DTYPE == "bf16":
        import ml_dtypes

        mmd_np = ml_dtypes.bfloat16
    else:
        mmd_np = np.float32

    def cvt(a):
        return np.ascontiguousarray(a).astype(mmd_np)

    kk = np.arange(128)[:, None]
    qq = np.arange(128)[None, :]
    # scores carry 2^32 (each of qt/kt carries 2^16); the mask add must be in
    # the same scaled units (folded back out by the Exp scale argument)
    mtri = (kk > qq).astype(np.float32) * (NEG * 2.0 ** 32)
    ident = np.eye(128, dtype=np.float32)

    import ml_dtypes as mld

    def pack8(a):
        """[D, n] -> fp8 DoubleRow layout [128, 4, 2, n]."""
        a = np.asarray(a, np.float32).reshape(4, 2, 128, -1)
        a = np.clip(a, -240.0, 240.0).transpose(2, 0, 1, 3)
        return np.ascontiguousarray(a).astype(mld.float8_e4m3)

    WS = 256.0   # fp8 weight pre-scale (2^8)

    in_maps = []
    for c in range(NCORES):
        b, g = divmod(c, 2)
        sl = slice(g * GCH, (g + 1) * GCH)
        im = {
            "x8q": pack8(q[b].T),
            "x8k": pack8(k[b].T),
            "xvT": cvt(v[b].T),
            "w18_q": pack8(w["wq1"][sl].T * WS),
            # fold the 1/sqrt(dk) score scale into the non-silu Q branch,
            # and 0.5 everywhere (silu computed as A*(1+tanh(A/2)) = 2*silu)
            "w28_q": pack8(w["wq2"][sl].T * (scale * 0.5 * WS)),
            "w18_k": pack8(w["wk1"][sl].T * WS),
            "w28_k": pack8(w["wk2"][sl].T * (0.5 * WS)),
            "w1T_v": cvt(w["wv1"][sl].T),
            "w2T_v": cvt(w["wv2"][sl].T * 0.5),
            "b1_q": np.ascontiguousarray(
                (bias["bq1"][sl] * WS).reshape(4, 128).T),
            "b1h_q": np.ascontiguousarray(
                (bias["bq1"][sl] * 0.5).reshape(4, 128).T),
            "b2_q": np.ascontiguousarray(
                (bias["bq2"][sl] * (scale * 0.5 * WS)).reshape(4, 128).T),
            "b1_k": np.ascontiguousarray(
                (bias["bk1"][sl] * WS).reshape(4, 128).T),
            "b1h_k": np.ascontiguousarray(
                (bias["bk1"][sl] * 0.5).reshape(4, 128).T),
            "b2_k": np.ascontiguousarray(
                (bias["bk2"][sl] * (0.5 * WS)).reshape(4, 128).T),
            "b1_v": cvt(bias["bv1"][sl].reshape(1, GCH)),
            "b2_v": cvt((bias["bv2"][sl] * 0.5).reshape(1, GCH)),
            "woT": cvt(
                w["wo"][:, sl].T.reshape(4, 128, D).transpose(1, 0, 2)),
            "ident": cvt(ident),
        }
        if mask_mode == "causal":
            im["mtri"] = cvt(mtri)
        elif mask_mode == "general":
            im["m01T"] = cvt(m01T)
        in_maps.append(im)
    return mask_mode, in_maps, bias["bo"]


LAST_NC = None


def kernel(**inputs):
    global LAST_RESULT, LAST_NC
    mask_mode, in_maps, bo = _host_prepare(inputs)
    nc = build_program(mask_mode)
    LAST_NC = nc

    import concourse.bass_utils as bu

    if TRACE:
        import types

        try:
            from trn_agent_boot.trn_boot import _ntff_profile_via_ctypes

            hook = _ntff_profile_via_ctypes("/opt/axon/libaxon_pjrt.so")
            m = types.ModuleType("antenv.axon_hooks")
            m.get_axon_ntff_profile_hook = lambda: hook
            import antenv  # noqa: F401

            sys.modules["antenv.axon_hooks"] = m
            bu.upload_artifacts = lambda d: "local://skipped"
        except Exception as e:
            print("profiling hook install failed:", e)

    res = bu.run_bass_kernel_spmd(
        nc, in_maps, core_ids=list(range(NCORES)),
        trace=TRACE, trace_cores=TRACE_CORES,
    )
    LAST_RESULT = res

    out = np.empty((B, S, D), dtype=np.float32)
    for b in range(B):
        out[b] = (res.results[2 * b]["pout"] + res.results[2 * b + 1]["pout"]
                  + bo[None, :])
    return out



# revision 56
# speedup vs baseline: 1.1847x; 1.0204x over previous
"""SwiGLU-projected causal MHA (B=4, S=2048, D=1024, H=16) on 8 TRN2 NeuronCores.

Sharding: core c -> (batch b = c//2, head-group g = c%2).  Each core computes
the SwiGLU Q/K/V projections for its 512 output channels (= 8 heads) of its
batch, runs causal attention for those heads, and produces a partial output
projection (contraction over its 512 channels).  The host sums the two
partials per batch and adds the output bias.

v3: projections and attention are software-pipelined per 512-seq slice:
after projecting q/k/v slice t, attention for query group qg=t runs; its
exp/normalization work (ACT/DVE) overlaps the next slice's projection
matmuls, keeping the PE dense (and its HAM clock at 2.4 GHz).  Projection
pairs and score pairs share one 3-slot PSUM pool so whichever phase is
active gets the double-buffering.

Attention per (qg, pj=head pair): scores S^T [k-part, q-free] for both heads
go to one 2-bank PSUM pair tile; causal masking is PE-side (an identity
matmul accumulates -1e4*triu onto the diagonal 128x128 subtile, and fully
masked column ranges are never computed).  One Exp covers both heads
straight out of PSUM.  For qg>=1 the exp output is written as fp8e4 and the
AV matmuls contract kc pairs with perf_mode=DoubleRow against an fp8 copy
of V (2x PE rate); qg=0 (queries with few visible keys, where quantization
noise does not average out) keeps a bf16 AV path.  V carries a ones column
so the AV emits softmax denominators at output row 64; they are staged at
partitions 0/32 of a persistent tile, inverted with one
reciprocal_approx_fast, and broadcast to 64 rows via K=1 ones matmuls into
the free upper rows of the ctx PSUM banks.
"""
import sys

sys.path.insert(0, "/opt/trn_rl_repo")
import numpy as np

import concourse.bacc as bacc
import concourse.tile as tile
import concourse.mybir as mybir

B, S, D = 4, 2048, 1024
H, DK = 16, 64
NCORES = 8
GCH = 512          # channels per core (8 heads)
NT = S // 128      # 16 seq chunks
F32 = mybir.dt.float32
FP8 = mybir.dt.float8e4
ACTF = mybir.ActivationFunctionType
ALU = mybir.AluOpType
PERF = mybir.MatmulPerfMode
NEG = -10000.0     # additive causal mask (exp(x-1e4) == 0 in fp32)

TRACE = False          # set by test.py for profiling runs
TRACE_CORES = None
LAST_RESULT = None     # BassKernelResults stash for test.py
MM_DTYPE = "bf16"
USE_FP8_AV = True      # fp8 DoubleRow AV for qg>=1 (causal mode only)


def build_program(mask_mode):
    """mask_mode: 'causal' (tril), 'full' (all ones), 'general' (arbitrary)."""
    MMD = mybir.dt.bfloat16 if MM_DTYPE == "bf16" else mybir.dt.float32r
    fp8_av = USE_FP8_AV and mask_mode == "causal"
    nc = bacc.Bacc("TRN2", target_bir_lowering=False, debug=False)

    # q/k projections run as fp8 DoubleRow: x and w pre-packed on the host as
    # [128 part, 4 dc-pair, 2 slot, n] with contraction index 256*m+128*s+p.
    # Weights carry 2^8 (fp8 subnormal avoidance), so qt/kt carry 2^16 and
    # raw scores carry 2^32 -- folded out via the Exp scale argument.
    x8_d = {s: nc.dram_tensor(f"x8{s}", [128, 4, 2, S], FP8,
                              kind="ExternalInput") for s in "qk"}
    w8_d = {f"{wn}{s}": nc.dram_tensor(f"{wn}8_{s}", [128, 4, 2, GCH], FP8,
                                       kind="ExternalInput")
            for s in "qk" for wn in ("w1", "w2")}
    xT = {"v": nc.dram_tensor("xvT", [D, S], MMD, kind="ExternalInput")}
    w1T = {"v": nc.dram_tensor("w1T_v", [D, GCH], MMD, kind="ExternalInput")}
    w2T = {"v": nc.dram_tensor("w2T_v", [D, GCH], MMD, kind="ExternalInput")}
    EXPS = 2.0 ** -32
    bias_d = {}
    for s in "qk":
        for bn in ("b1", "b2", "b1h"):
            bias_d[f"{bn}_{s}"] = nc.dram_tensor(f"{bn}_{s}", [128, 4], F32,
                                                 kind="ExternalInput")
    b1v_d = nc.dram_tensor("b1_v", [1, GCH], MMD, kind="ExternalInput")
    b2v_d = nc.dram_tensor("b2_v", [1, GCH], MMD, kind="ExternalInput")
    woT_d = nc.dram_tensor("woT", [128, 4, D], MMD, kind="ExternalInput")
    mtri_d = m01T_d = None
    if mask_mode == "causal":
        mtri_d = nc.dram_tensor("mtri", [128, 128], MMD, kind="ExternalInput")
    elif mask_mode == "general":
        m01T_d = nc.dram_tensor("m01T", [S, S], MMD, kind="ExternalInput")
    ident_d = nc.dram_tensor("ident", [128, 128], MMD, kind="ExternalInput")
    pout_d = nc.dram_tensor("pout", [S, D], F32, kind="ExternalOutput")

    def kc_count(qg):
        return 4 * qg + 4 if mask_mode == "causal" else NT

    def col0(qg, kc):
        """first valid q column (within the 512 q group) for key block kc."""
        if mask_mode != "causal":
            return 0
        i = kc - 4 * qg
        return 0 if i < 0 else 128 * i

    with tile.TileContext(nc) as tc:
        with (
            tc.tile_pool(name="persist", bufs=1) as persist,
            tc.tile_pool(name="xpool", bufs=10) as xpool,
            tc.tile_pool(name="xpool8", bufs=12) as xpool8,
            tc.tile_pool(name="stage", bufs=3) as stage,
            tc.tile_pool(name="apool", bufs=4) as apool,
            tc.tile_pool(name="apoolb", bufs=2) as apoolb,
            tc.tile_pool(name="smalls", bufs=2) as smalls,
            tc.tile_pool(name="ctpool", bufs=3) as ctpool,
            tc.tile_pool(name="ostage", bufs=2) as ostage,
            tc.tile_pool(name="mpool", bufs=1) as mpool,
            tc.tile_pool(name="work", bufs=3, space="PSUM") as work,
            tc.tile_pool(name="cxps", bufs=2, space="PSUM") as cxps,
        ):
            # ---------------- persistent state ----------------
            qt_t = [persist.tile([128, 4, 512], MMD, tag=f"qt{t}", name=f"qt{t}")
                    for t in range(4)]
            kt_t = [persist.tile([128, 4, 512], MMD, tag=f"kt{t}", name=f"kt{t}")
                    for t in range(4)]
            # bf16 V (per 512-slice) and, in causal mode, an fp8 copy padded
            # to 80 so the DoubleRow kc-pair step stays 16-aligned
            vb_t = [persist.tile([128, 4, 8, 65], MMD, tag=f"vb{t}", name=f"vb{t}")
                    for t in range(4)]
            v8_t = None
            if fp8_av:
                v8_t = [persist.tile([128, 8, 4, 80], FP8, tag=f"v8{t}", name=f"v8{t}")
                        for t in range(4)]
            woT_sb = persist.tile([128, 4, D], MMD, tag="wo")
            onesf = persist.tile([1, 128], F32, tag="onesf")
            ones_r = persist.tile([1, 128], MMD, tag="ones_r")
            onescol = persist.tile([128, 1], F32, tag="onescol")
            ident_sb = persist.tile([128, 128], MMD, tag="ident")
            nc.sync.dma_start(ident_sb[:], ident_d[:])
            if mask_mode == "causal":
                mtri_sb = persist.tile([128, 128], MMD, tag="mtri")
                nc.sync.dma_start(mtri_sb[:], mtri_d[:])
            den_sb = persist.tile([33, 512], F32, tag="den")
            ones33f = persist.tile([33, 128], F32, tag="ones33f")
            ones33 = persist.tile([33, 128], MMD, tag="ones33")

            def deferred_init():
                nc.any.memset(onesf[:], 1.0)
                nc.vector.tensor_copy(ones_r[:], onesf[:])
                nc.any.memset(onescol[:], 1.0)
                for t in range(4):
                    nc.gpsimd.memset(vb_t[t][:, :, :, 64:65], 1.0)
                    if fp8_av:
                        nc.gpsimd.memset(v8_t[t][:, :, :, 64:65], 1.0)
                nc.gpsimd.memset(den_sb[:], 1.0)
                nc.any.memset(ones33f[:], 1.0)
                nc.vector.tensor_copy(ones33[:], ones33f[:])

            # projection weights stay resident; DMA them lazily at first use
            wsb = {}
            bsb = {}

            def load_weights(s):
                if s in "qk":
                    for wn in ("w1", "w2"):
                        wt = persist.tile([128, 4, 2, GCH], FP8,
                                          tag=f"{wn}{s}", name=f"{wn}{s}")
                        nc.gpsimd.dma_start(wt[:], w8_d[f"{wn}{s}"][:])
                        wsb[f"{wn}{s}"] = wt
                else:
                    for wn, wd in (("w1", w1T[s]), ("w2", w2T[s])):
                        wt = persist.tile([128, 8, GCH], MMD, tag=f"{wn}{s}",
                                          name=f"{wn}{s}")
                        for dc in range(8):
                            nc.gpsimd.dma_start(
                                wt[:, dc, :], wd[dc * 128:(dc + 1) * 128, :]
                            )
                        wsb[f"{wn}{s}"] = wt
                if s != "v":
                    for bn in ("b1", "b2", "b1h"):
                        bt = persist.tile([128, 4], F32, tag=f"{bn}{s}",
                                          name=f"{bn}{s}")
                        nc.sync.dma_start(bt[:], bias_d[f"{bn}_{s}"][:])
                        bsb[f"{bn}{s}"] = bt
                else:
                    bsb["b1v"] = b1vr = persist.tile([1, GCH], MMD, tag="b1v",
                                                     name="b1v")
                    bsb["b2v"] = b2vr = persist.tile([1, GCH], MMD, tag="b2v",
                                                     name="b2v")
                    nc.sync.dma_start(b1vr[:], b1v_d[:])
                    nc.sync.dma_start(b2vr[:], b2v_d[:])

            # warm the PE HAM clock while the first weight/x DMAs land; the
            # warmup weights come from a memset (not a DMA) so the first
            # matmul issues as early as possible
            wu_w = persist.tile([128, 128], MMD, tag="wu_w")
            nc.gpsimd.memset(wu_w[:], 0.0)
            wu = work.tile([128, 2, 512], F32, tag="wk", name="warmup")
            for i in range(96):
                nc.tensor.matmul(
                    wu[:, 0, 0:128], wu_w[:], wu_w[:],
                    start=True, stop=True, skip_group_check=True,
                )

            # ---------------- phase A: one 512-seq slice of s ----------------
            x_pending = {}

            def prefetch_x(s, t):
                if (s, t) in x_pending:
                    return
                xts = []
                if s in "qk":
                    for m in range(4):
                        xt = xpool8.tile([128, 2, 512], FP8, tag="xt8")
                        nc.sync.dma_start(
                            xt[:],
                            x8_d[s][:, m, :, t * 512:(t + 1) * 512],
                        )
                        xts.append(xt)
                else:
                    for dc in range(8):
                        xt = xpool.tile([128, 512], MMD, tag="xt")
                        nc.sync.dma_start(
                            xt[:],
                            xT[s][dc * 128:(dc + 1) * 128,
                                  t * 512:(t + 1) * 512],
                        )
                        xts.append(xt)
                x_pending[(s, t)] = xts

            def project_units(s, t):
                if t == 0:
                    load_weights(s)
                if s == "q" and t == 1:
                    nc.sync.dma_start(woT_sb[:], woT_d[:])
                prefetch_x(s, t)
                xts = x_pending.pop((s, t))
                for j in range(4):
                    pr = work.tile([128, 2, 512], F32, tag="wk")
                    if s == "v":
                        for dc in range(8):
                            # seq on partitions: lhsT = x chunk
                            nc.tensor.matmul(
                                pr[:, 0, :],
                                xts[dc][:, j * 128:(j + 1) * 128],
                                wsb["w1v"][:, dc, :],
                                start=(dc == 0), stop=False,
                            )
                            nc.tensor.matmul(
                                pr[:, 1, :],
                                xts[dc][:, j * 128:(j + 1) * 128],
                                wsb["w2v"][:, dc, :],
                                start=(dc == 0), stop=False,
                            )
                    else:
                        for m in range(4):
                            nc.tensor.matmul(
                                pr[:, 0, :],
                                wsb[f"w1{s}"][:, m, :,
                                              j * 128:(j + 1) * 128],
                                xts[m][:],
                                start=(m == 0), stop=(m == 3),
                                perf_mode=PERF.DoubleRow,
                            )
                            nc.tensor.matmul(
                                pr[:, 1, :],
                                wsb[f"w2{s}"][:, m, :,
                                              j * 128:(j + 1) * 128],
                                xts[m][:],
                                start=(m == 0), stop=(m == 3),
                                perf_mode=PERF.DoubleRow,
                            )
                    act = stage.tile([128, 512], F32, tag="act")
                    if s == "v":
                        # fold the biases into the accumulation (they vary
                        # along the free/channel dim)
                        nc.tensor.matmul(pr[:, 0, :], ones_r[:], bsb["b1v"][:],
                                         start=False, stop=True)
                        nc.tensor.matmul(pr[:, 1, :], ones_r[:], bsb["b2v"][:],
                                         start=False, stop=True)
                        nc.scalar.activation(act[:], pr[:, 0, :],
                                             ACTF.Tanh, scale=0.5)
                        u = stage.tile([128, 512], F32, tag="u")
                        # silu2(A) = (tanh(A/2) + 1) * A in one op
                        nc.vector.scalar_tensor_tensor(
                            u[:], act[:], 1.0, pr[:, 0, :],
                            op0=ALU.add, op1=ALU.mult,
                        )
                        src1 = pr[:, 1, :].rearrange("p (h d) -> p h d", h=8)
                        src2 = u[:].rearrange("p (h d) -> p h d", h=8)
                        nc.vector.tensor_tensor(
                            vb_t[t][:, j, :, 0:64], src1, src2, ALU.mult
                        )
                        if fp8_av:
                            with nc.allow_low_precision(reason="fp8 AV copy"):
                                nc.gpsimd.tensor_copy(
                                    v8_t[t][:, :, j, 0:64],
                                    vb_t[t][:, j, :, 0:64],
                                )
                    else:
                        bias1 = bsb[f"b1{s}"][:, j:j + 1]
                        bias2 = bsb[f"b2{s}"][:, j:j + 1]
                        # q/k PSUM carries 2^8 (fp8 weight pre-scale); fold
                        # the unscale into the tanh affine (b1h is unscaled)
                        nc.scalar.activation(
                            act[:], pr[:, 0, :], ACTF.Tanh,
                            scale=0.5 / 256.0,
                            bias=bsb[f"b1h{s}"][:, j:j + 1],
                        )
                        a_sb = stage.tile([128, 512], F32, tag="u")
                        nc.vector.tensor_scalar_add(a_sb[:], pr[:, 0, :],
                                                    bias1)
                        nc.vector.scalar_tensor_tensor(
                            act[:], act[:], 1.0, a_sb[:],
                            op0=ALU.add, op1=ALU.mult,
                        )
                        dst = (qt_t if s == "q" else kt_t)[t][:, j, :]
                        nc.vector.scalar_tensor_tensor(
                            dst, pr[:, 1, :], bias2, act[:],
                            op0=ALU.add, op1=ALU.mult,
                        )
                    yield

            def project(s, t):
                for _ in project_units(s, t):
                    pass

            # ---------------- phase B: one head pair of one query group -----
            ct_tiles = {}
            ctx_of = {}

            def attention_pair_units(qg, pj, mtiles):
                kcmax = kc_count(qg)
                use8 = fp8_av and qg >= 1
                if pj == 0:
                    ct_tiles[qg] = ctpool.tile([128, 4, 512], MMD, tag="ct",
                                               name=f"ct{qg}")
                ctx = [cxps.tile([128, 512], F32, tag="cx", name=f"ctx_{i}")
                       for i in range(2)]
                ctx_of[(qg, pj)] = ctx
                npair = (kcmax + 1) // 2
                for p in range(npair):
                    attn = (apool if use8 else apoolb).tile(
                        [128, 2, 2, 512], FP8 if use8 else MMD,
                        tag="at8" if use8 else "atb", name=f"at_{qg}_{pj}_{p}",
                    )
                    kcs = [kc for kc in (2 * p, 2 * p + 1) if kc < kcmax]
                    diag_adds = []
                    sc_of = {}
                    for kc in kcs:
                        c0 = col0(qg, kc)
                        tt, lkc = divmod(kc, 4)
                        diag = mask_mode == "causal" and kc >= 4 * qg
                        sc = work.tile([128, 2, 512], F32, tag="wk",
                                           name=f"sc_{qg}_{pj}_{kc}")
                        sc_of[kc] = sc
                        for par in range(2):
                            bp = par * 64
                            nc.tensor.matmul(
                                sc[:, par, c0:],
                                kt_t[tt][bp:bp + 64, pj,
                                         lkc * 128:(lkc + 1) * 128],
                                qt_t[qg][bp:bp + 64, pj, c0:],
                                start=True, stop=not diag,
                            )
                        if diag:
                            diag_adds.append((sc, c0))
                    # batched mask adds: mtri weights stay loaded across all
                    # diag subtiles of the pair (avoids kt<->mtri LDW thrash)
                    for sc, c0 in diag_adds:
                        for par in range(2):
                            nc.tensor.matmul(
                                sc[:, par, c0:c0 + 128],
                                ident_sb[:],
                                mtri_sb[:],
                                start=False, stop=True,
                            )
                    for kc in kcs:
                        c0 = col0(qg, kc)
                        sc = sc_of[kc]
                        with nc.allow_low_precision(reason="attn fp8"):
                            nc.scalar.activation(
                                attn[:, kc & 1, :, c0:], sc[:, :, c0:],
                                ACTF.Exp, scale=EXPS,
                            )
                        if mask_mode == "general":
                            for par in range(2):
                                nc.vector.tensor_tensor(
                                    attn[:, kc & 1, par, :],
                                    attn[:, kc & 1, par, :],
                                    mtiles[kc], ALU.mult,
                                )
                    # ---- AV ----
                    first = (p == 0)
                    last = (p == npair - 1)
                    anydiag = any(mask_mode == "causal" and kc >= 4 * qg
                                  for kc in kcs)
                    for par in range(2):
                        hl = 2 * pj + par
                        if use8 and not anydiag and len(kcs) == 2:
                            tt, l0 = divmod(2 * p, 4)
                            nc.tensor.matmul(
                                ctx[par][0:65, :],
                                v8_t[tt][:, hl, l0:l0 + 2, 0:65],
                                attn[:, :, par, :],
                                start=first, stop=last,
                                perf_mode=PERF.DoubleRow,
                                skip_group_check=True,
                            )
                        else:
                            for kc in kcs:
                                c0 = col0(qg, kc)
                                tt, lkc = divmod(kc, 4)
                                if use8:
                                    vt = v8_t[tt][:, hl, lkc, 0:65]
                                else:
                                    vt = vb_t[tt][:, lkc, hl, :]
                                nc.tensor.matmul(
                                    ctx[par][0:65, c0:],
                                    vt,
                                    attn[:, kc & 1, par, c0:],
                                    start=(first and kc == kcs[0]),
                                    stop=(last and kc == kcs[-1]),
                                    skip_group_check=True,
                                )
                    yield

            def normalize_unit(qg, pj):
                # ---- normalize both heads of the pair into ct_qg ----
                ct_qg = ct_tiles[qg]
                ctx = ctx_of.pop((qg, pj))
                for par in range(2):
                    nc.vector.tensor_copy(
                        den_sb[32 * par:32 * par + 1, :],
                        ctx[par][64:65, :],
                    )
                rec = smalls.tile([33, 512], F32, tag="rec")
                nc.vector.reciprocal_approx_fast(rec[:], den_sb[:])
                rec_b = smalls.tile([33, 512], MMD, tag="recb")
                nc.vector.tensor_copy(rec_b[:], rec[:])
                # broadcast each reciprocal row into the free upper rows
                # (64:128) of its own ctx PSUM bank
                for par in range(2):
                    nc.tensor.matmul(
                        ctx[par][64:128, :],
                        ones33[32 * par:32 * par + 1, 0:64],
                        rec_b[32 * par:32 * par + 1, :],
                        start=True, stop=True,
                        tile_position=(32 * par, 64),
                        skip_group_check=True,
                    )
                bc_sb = smalls.tile([128, 512], F32, tag="bcs")
                for par in range(2):
                    nc.vector.tensor_copy(
                        bc_sb[64 * par:64 * par + 64, :],
                        ctx[par][64:128, :],
                    )
                for par in range(2):
                    bp = par * 64
                    nc.vector.tensor_tensor(
                        ct_qg[bp:bp + 64, pj, :],
                        ctx[par][0:64, :],
                        bc_sb[64 * par:64 * par + 64, :],
                        ALU.mult,
                    )

            def attention_part(qg, pj, mtiles):
                for _ in attention_pair_units(qg, pj, mtiles):
                    pass
                normalize_unit(qg, pj)

            def attention_chain(qg, mtiles):
                for pj in range(4):
                    for _ in attention_pair_units(qg, pj, mtiles):
                        yield
                    normalize_unit(qg, pj)
                    yield

            def proj_stream(s, t, prefetches=()):
                for j, _ in enumerate(project_units(s, t)):
                    if j == 1:
                        for nx in prefetches:
                            prefetch_x(*nx)
                    yield

            def proj_chain(t):
                nxt = {"q": [("k", t)], "k": [("v", t)],
                       "v": [("q", t + 1)] if t < 3 else []}
                for s in "qkv":
                    yield from proj_stream(s, t, nxt[s])

            def weave(main_gen, side_gen, per_unit):
                """emit per_unit side units after each main unit (fractional
                credits), then drain whatever remains."""
                credit = 0.0
                for _ in main_gen:
                    credit += per_unit
                    while credit >= 1.0:
                        if next(side_gen, None) is None:
                            credit = 0.0
                            break
                        credit -= 1.0
                for _ in side_gen:
                    pass

            def attention_out_units(qg):
                ct_qg = ct_tiles[qg]
                for ns in range(4):
                    nt_i = qg * 4 + ns
                    nsl = slice(ns * 128, (ns + 1) * 128)
                    po = work.tile([128, 2, 512], F32, tag="wk",
                                   name=f"po_{qg}_{ns}")
                    for oh in range(2):
                        for j in range(4):
                            nc.tensor.matmul(
                                po[:, oh, :],
                                ct_qg[:, j, nsl],
                                woT_sb[:, j, oh * 512:(oh + 1) * 512],
                                start=(j == 0), stop=(j == 3),
                            )
                    ot = ostage.tile([128, 1024], F32, tag="ot")
                    nc.vector.tensor_copy(ot[:], po[:])
                    nc.sync.dma_start(
                        pout_d[nt_i * 128:(nt_i + 1) * 128, :],
                        ot[:],
                    )
                    yield

            def attention_out(qg):
                for _ in attention_out_units(qg):
                    pass

            def general_mtiles(qg):
                if mask_mode != "general":
                    return None
                kcmax = kc_count(qg)
                mtiles = []
                mt_sb = mpool.tile([128, NT, 512], MMD, tag="mt")
                for kc in range(kcmax):
                    nc.sync.dma_start(
                        mt_sb[:, kc, :],
                        m01T_d[kc * 128:(kc + 1) * 128,
                               qg * 512:(qg + 1) * 512],
                    )
                    mtiles.append(mt_sb[:, kc, :])
                return mtiles

            # ---------------- interleaved schedule ----------------
            # Unit-level weave: attention(qg=t-1)'s kc-pair units are the main
            # stream; the three projection streams of slice t (plus, from t=2,
            # the output projection of group t-2) are the side stream.  The
            # fine interleave keeps proj matmuls between attention pairs so
            # the PE never outruns ACT's exp stream (which is slower per kc
            # pair than the PE work it gates), and the HAM clock stays warm.
            deferred_init()
            if mask_mode == "causal":
                import itertools

                for _ in proj_chain(0):
                    pass
                for t in range(1, 4):
                    qg = t - 1
                    main = attention_chain(qg, None)
                    side = proj_chain(t)
                    n_main = 4 * (((kc_count(qg) + 1) // 2) + 1)
                    n_side = 12
                    if qg == 2:
                        side = itertools.chain(side, attention_out_units(0))
                        n_side += 4
                    weave(main, side, n_side / n_main)
                main = attention_chain(3, None)
                side = itertools.chain(attention_out_units(1),
                                       attention_out_units(2))
                weave(main, side, 8 / 36)
                attention_out(3)
            else:
                for t in range(4):
                    for s in "qkv":
                        project(s, t)
                for qg in range(4):
                    mtiles = general_mtiles(qg)
                    for pj in range(4):
                        attention_part(qg, pj, mtiles)
                    attention_out(qg)
    nc.compile()
    return nc


def _host_prepare(inputs):
    """Split the full problem into 8 per-core input maps + host-side info."""
    q = np.asarray(inputs["query"], dtype=np.float32)
    k = np.asarray(inputs["key"], dtype=np.float32)
    v = np.asarray(inputs["value"], dtype=np.float32)
    mask = np.asarray(inputs["mask"])
    w = {n: np.asarray(inputs[n], dtype=np.float32)
         for n in ("wq1", "wq2", "wk1", "wk2", "wv1", "wv2", "wo")}
    bias = {n: np.asarray(inputs[n], dtype=np.float32)
            for n in ("bq1", "bq2", "bk1", "bk2", "bv1", "bv2", "bo")}

    m = mask.reshape(S, S)
    if np.array_equal(m != 0, np.tril(np.ones((S, S), bool))):
        mask_mode = "causal"
    elif np.all(m != 0):
        mask_mode = "full"
    else:
        mask_mode = "general"

    m01T = None
    if mask_mode == "general":
        m01T = np.ascontiguousarray((m != 0).T.astype(np.float32))

    scale = 1.0 / np.sqrt(DK).astype(np.float32)

    if MM_DTYPE == "bf16":
        import ml_dtypes

        mmd_np = ml_dtypes.bfloat16
    else:
        mmd_np = np.float32

    def cvt(a):
        return np.ascontiguousarray(a).astype(mmd_np)

    kk = np.arange(128)[:, None]
    qq = np.arange(128)[None, :]
    # scores carry 2^32 (each of qt/kt carries 2^16); the mask add must be in
    # the same scaled units (folded back out by the Exp scale argument)
    mtri = (kk > qq).astype(np.float32) * (NEG * 2.0 ** 32)
    ident = np.eye(128, dtype=np.float32)

    import ml_dtypes as mld

    def pack8(a):
        """[D, n] -> fp8 DoubleRow layout [128, 4, 2, n]."""
        a = np.asarray(a, np.float32).reshape(4, 2, 128, -1)
        a = np.clip(a, -240.0, 240.0).transpose(2, 0, 1, 3)
        return np.ascontiguousarray(a).astype(mld.float8_e4m3)

    WS = 256.0   # fp8 weight pre-scale (2^8)

    in_maps = []
    for c in range(NCORES):
        b, g = divmod(c, 2)
        sl = slice(g * GCH, (g + 1) * GCH)
        im = {
            "x8q": pack8(q[b].T),
            "x8k": pack8(k[b].T),
            "xvT": cvt(v[b].T),
            "w18_q": pack8(w["wq1"][sl].T * WS),
            # fold the 1/sqrt(dk) score scale into the non-silu Q branch,
            # and 0.5 everywhere (silu computed as A*(1+tanh(A/2)) = 2*silu)
            "w28_q": pack8(w["wq2"][sl].T * (scale * 0.5 * WS)),
            "w18_k": pack8(w["wk1"][sl].T * WS),
            "w28_k": pack8(w["wk2"][sl].T * (0.5 * WS)),
            "w1T_v": cvt(w["wv1"][sl].T),
            "w2T_v": cvt(w["wv2"][sl].T * 0.5),
            "b1_q": np.ascontiguousarray(
                (bias["bq1"][sl] * WS).reshape(4, 128).T),
            "b1h_q": np.ascontiguousarray(
                (bias["bq1"][sl] * 0.5).reshape(4, 128).T),
            "b2_q": np.ascontiguousarray(
                (bias["bq2"][sl] * (scale * 0.5 * WS)).reshape(4, 128).T),
            "b1_k": np.ascontiguousarray(
                (bias["bk1"][sl] * WS).reshape(4, 128).T),
            "b1h_k": np.ascontiguousarray(
                (bias["bk1"][sl] * 0.5).reshape(4, 128).T),
            "b2_k": np.ascontiguousarray(
                (bias["bk2"][sl] * (0.5 * WS)).reshape(4, 128).T),
            "b1_v": cvt(bias["bv1"][sl].reshape(1, GCH)),
            "b2_v": cvt((bias["bv2"][sl] * 0.5).reshape(1, GCH)),
            "woT": cvt(
                w["wo"][:, sl].T.reshape(4, 128, D).transpose(1, 0, 2)),
            "ident": cvt(ident),
        }
        if mask_mode == "causal":
            im["mtri"] = cvt(mtri)
        elif mask_mode == "general":
            im["m01T"] = cvt(m01T)
        in_maps.append(im)
    return mask_mode, in_maps, bias["bo"]


LAST_NC = None


def kernel(**inputs):
    global LAST_RESULT, LAST_NC
    mask_mode, in_maps, bo = _host_prepare(inputs)
    nc = build_program(mask_mode)
    LAST_NC = nc

    import concourse.bass_utils as bu

    if TRACE:
        import types

        try:
            from trn_agent_boot.trn_boot import _ntff_profile_via_ctypes

            hook = _ntff_profile_via_ctypes("/opt/axon/libaxon_pjrt.so")
            m = types.ModuleType("antenv.axon_hooks")
            m.get_axon_ntff_profile_hook = lambda: hook
            import antenv  # noqa: F401

            sys.modules["antenv.axon_hooks"] = m
            bu.upload_artifacts = lambda d: "local://skipped"
        except Exception as e:
            print("profiling hook install failed:", e)

    res = bu.run_bass_kernel_spmd(
        nc, in_maps, core_ids=list(range(NCORES)),
        trace=TRACE, trace_cores=TRACE_CORES,
    )
    LAST_RESULT = res

    out = np.empty((B, S, D), dtype=np.float32)
    for b in range(B):
        out[b] = (res.results[2 * b]["pout"] + res.results[2 * b + 1]["pout"]
                  + bo[None, :])
    return out



# revision 57
# speedup vs baseline: 1.1907x; 1.0050x over previous
"""SwiGLU-projected causal MHA (B=4, S=2048, D=1024, H=16) on 8 TRN2 NeuronCores.

Sharding: core c -> (batch b = c//2, head-group g = c%2).  Each core computes
the SwiGLU Q/K/V projections for its 512 output channels (= 8 heads) of its
batch, runs causal attention for those heads, and produces a partial output
projection (contraction over its 512 channels).  The host sums the two
partials per batch and adds the output bias.

v3: projections and attention are software-pipelined per 512-seq slice:
after projecting q/k/v slice t, attention for query group qg=t runs; its
exp/normalization work (ACT/DVE) overlaps the next slice's projection
matmuls, keeping the PE dense (and its HAM clock at 2.4 GHz).  Projection
pairs and score pairs share one 3-slot PSUM pool so whichever phase is
active gets the double-buffering.

Attention per (qg, pj=head pair): scores S^T [k-part, q-free] for both heads
go to one 2-bank PSUM pair tile; causal masking is PE-side (an identity
matmul accumulates -1e4*triu onto the diagonal 128x128 subtile, and fully
masked column ranges are never computed).  One Exp covers both heads
straight out of PSUM.  For qg>=1 the exp output is written as fp8e4 and the
AV matmuls contract kc pairs with perf_mode=DoubleRow against an fp8 copy
of V (2x PE rate); qg=0 (queries with few visible keys, where quantization
noise does not average out) keeps a bf16 AV path.  V carries a ones column
so the AV emits softmax denominators at output row 64; they are staged at
partitions 0/32 of a persistent tile, inverted with one
reciprocal_approx_fast, and broadcast to 64 rows via K=1 ones matmuls into
the free upper rows of the ctx PSUM banks.
"""
import sys

sys.path.insert(0, "/opt/trn_rl_repo")
import numpy as np

import concourse.bacc as bacc
import concourse.tile as tile
import concourse.mybir as mybir

B, S, D = 4, 2048, 1024
H, DK = 16, 64
NCORES = 8
GCH = 512          # channels per core (8 heads)
NT = S // 128      # 16 seq chunks
F32 = mybir.dt.float32
FP8 = mybir.dt.float8e4
ACTF = mybir.ActivationFunctionType
ALU = mybir.AluOpType
PERF = mybir.MatmulPerfMode
NEG = -10000.0     # additive causal mask (exp(x-1e4) == 0 in fp32)

TRACE = False          # set by test.py for profiling runs
TRACE_CORES = None
LAST_RESULT = None     # BassKernelResults stash for test.py
MM_DTYPE = "bf16"
USE_FP8_AV = True      # fp8 DoubleRow AV for qg>=1 (causal mode only)


def build_program(mask_mode):
    """mask_mode: 'causal' (tril), 'full' (all ones), 'general' (arbitrary)."""
    MMD = mybir.dt.bfloat16 if MM_DTYPE == "bf16" else mybir.dt.float32r
    fp8_av = USE_FP8_AV and mask_mode == "causal"
    nc = bacc.Bacc("TRN2", target_bir_lowering=False, debug=False)

    # q/k projections run as fp8 DoubleRow: x and w pre-packed on the host as
    # [128 part, 4 dc-pair, 2 slot, n] with contraction index 256*m+128*s+p.
    # Weights carry 2^8 (fp8 subnormal avoidance), so qt/kt carry 2^16 and
    # raw scores carry 2^32 -- folded out via the Exp scale argument.
    x8_d = {s: nc.dram_tensor(f"x8{s}", [128, 4, 2, S], FP8,
                              kind="ExternalInput") for s in "qk"}
    w8_d = {f"{wn}{s}": nc.dram_tensor(f"{wn}8_{s}", [128, 4, 2, GCH], FP8,
                                       kind="ExternalInput")
            for s in "qk" for wn in ("w1", "w2")}
    xT = {"v": nc.dram_tensor("xvT", [D, S], MMD, kind="ExternalInput")}
    w1T = {"v": nc.dram_tensor("w1T_v", [D, GCH], MMD, kind="ExternalInput")}
    w2T = {"v": nc.dram_tensor("w2T_v", [D, GCH], MMD, kind="ExternalInput")}
    EXPS = 2.0 ** -32
    bias_d = {}
    for s in "qk":
        for bn in ("b1", "b2", "b1h"):
            bias_d[f"{bn}_{s}"] = nc.dram_tensor(f"{bn}_{s}", [128, 4], F32,
                                                 kind="ExternalInput")
    b1v_d = nc.dram_tensor("b1_v", [1, GCH], MMD, kind="ExternalInput")
    b2v_d = nc.dram_tensor("b2_v", [1, GCH], MMD, kind="ExternalInput")
    woT_d = nc.dram_tensor("woT", [128, 4, D], MMD, kind="ExternalInput")
    mtri_d = m01T_d = None
    if mask_mode == "causal":
        mtri_d = nc.dram_tensor("mtri", [128, 128], MMD, kind="ExternalInput")
    elif mask_mode == "general":
        m01T_d = nc.dram_tensor("m01T", [S, S], MMD, kind="ExternalInput")
    ident_d = nc.dram_tensor("ident", [128, 128], MMD, kind="ExternalInput")
    pout_d = nc.dram_tensor("pout", [S, D], F32, kind="ExternalOutput")

    def kc_count(qg):
        return 4 * qg + 4 if mask_mode == "causal" else NT

    def col0(qg, kc):
        """first valid q column (within the 512 q group) for key block kc."""
        if mask_mode != "causal":
            return 0
        i = kc - 4 * qg
        return 0 if i < 0 else 128 * i

    with tile.TileContext(nc) as tc:
        with (
            tc.tile_pool(name="persist", bufs=1) as persist,
            tc.tile_pool(name="xpool", bufs=10) as xpool,
            tc.tile_pool(name="xpool8", bufs=12) as xpool8,
            tc.tile_pool(name="stage", bufs=3) as stage,
            tc.tile_pool(name="apool", bufs=4) as apool,
            tc.tile_pool(name="apoolb", bufs=2) as apoolb,
            tc.tile_pool(name="smalls", bufs=2) as smalls,
            tc.tile_pool(name="ctpool", bufs=3) as ctpool,
            tc.tile_pool(name="ostage", bufs=2) as ostage,
            tc.tile_pool(name="mpool", bufs=1) as mpool,
            tc.tile_pool(name="work", bufs=3, space="PSUM") as work,
            tc.tile_pool(name="cxps", bufs=2, space="PSUM") as cxps,
        ):
            # ---------------- persistent state ----------------
            qt_t = [persist.tile([128, 4, 512], MMD, tag=f"qt{t}", name=f"qt{t}")
                    for t in range(4)]
            kt_t = [persist.tile([128, 4, 512], MMD, tag=f"kt{t}", name=f"kt{t}")
                    for t in range(4)]
            # bf16 V (per 512-slice) and, in causal mode, an fp8 copy padded
            # to 80 so the DoubleRow kc-pair step stays 16-aligned
            vb_t = [persist.tile([128, 4, 8, 65], MMD, tag=f"vb{t}", name=f"vb{t}")
                    for t in range(4)]
            v8_t = None
            if fp8_av:
                v8_t = [persist.tile([128, 8, 4, 80], FP8, tag=f"v8{t}", name=f"v8{t}")
                        for t in range(4)]
            woT_sb = persist.tile([128, 4, D], MMD, tag="wo")
            onesf = persist.tile([1, 128], F32, tag="onesf")
            ones_r = persist.tile([1, 128], MMD, tag="ones_r")
            onescol = persist.tile([128, 1], F32, tag="onescol")
            ident_sb = persist.tile([128, 128], MMD, tag="ident")
            nc.sync.dma_start(ident_sb[:], ident_d[:])
            if mask_mode == "causal":
                mtri_sb = persist.tile([128, 128], MMD, tag="mtri")
                nc.sync.dma_start(mtri_sb[:], mtri_d[:])
            den_sb = persist.tile([33, 512], F32, tag="den")
            ones33f = persist.tile([33, 128], F32, tag="ones33f")
            ones33 = persist.tile([33, 128], MMD, tag="ones33")

            def deferred_init():
                nc.any.memset(onesf[:], 1.0)
                nc.vector.tensor_copy(ones_r[:], onesf[:])
                nc.any.memset(onescol[:], 1.0)
                for t in range(4):
                    nc.gpsimd.memset(vb_t[t][:, :, :, 64:65], 1.0)
                    if fp8_av:
                        nc.gpsimd.memset(v8_t[t][:, :, :, 64:65], 1.0)
                nc.gpsimd.memset(den_sb[:], 1.0)
                nc.any.memset(ones33f[:], 1.0)
                nc.vector.tensor_copy(ones33[:], ones33f[:])

            # projection weights stay resident; DMA them lazily at first use
            wsb = {}
            bsb = {}

            def load_weights(s):
                if s in "qk":
                    for wn in ("w1", "w2"):
                        wt = persist.tile([128, 4, 2, GCH], FP8,
                                          tag=f"{wn}{s}", name=f"{wn}{s}")
                        nc.gpsimd.dma_start(wt[:], w8_d[f"{wn}{s}"][:])
                        wsb[f"{wn}{s}"] = wt
                else:
                    for wn, wd in (("w1", w1T[s]), ("w2", w2T[s])):
                        wt = persist.tile([128, 8, GCH], MMD, tag=f"{wn}{s}",
                                          name=f"{wn}{s}")
                        for dc in range(8):
                            nc.gpsimd.dma_start(
                                wt[:, dc, :], wd[dc * 128:(dc + 1) * 128, :]
                            )
                        wsb[f"{wn}{s}"] = wt
                if s != "v":
                    for bn in ("b1", "b2", "b1h"):
                        bt = persist.tile([128, 4], F32, tag=f"{bn}{s}",
                                          name=f"{bn}{s}")
                        nc.sync.dma_start(bt[:], bias_d[f"{bn}_{s}"][:])
                        bsb[f"{bn}{s}"] = bt
                else:
                    bsb["b1v"] = b1vr = persist.tile([1, GCH], MMD, tag="b1v",
                                                     name="b1v")
                    bsb["b2v"] = b2vr = persist.tile([1, GCH], MMD, tag="b2v",
                                                     name="b2v")
                    nc.sync.dma_start(b1vr[:], b1v_d[:])
                    nc.sync.dma_start(b2vr[:], b2v_d[:])

            # warm the PE HAM clock while the first weight/x DMAs land; the
            # warmup weights come from a memset (not a DMA) so the first
            # matmul issues as early as possible
            wu_w = persist.tile([128, 128], MMD, tag="wu_w")
            nc.gpsimd.memset(wu_w[:], 0.0)
            wu = work.tile([128, 2, 512], F32, tag="wk", name="warmup")
            for i in range(96):
                nc.tensor.matmul(
                    wu[:, 0, 0:128], wu_w[:], wu_w[:],
                    start=True, stop=True, skip_group_check=True,
                )

            # ---------------- phase A: one 512-seq slice of s ----------------
            x_pending = {}

            def prefetch_x(s, t):
                if (s, t) in x_pending:
                    return
                xts = []
                if s in "qk":
                    for m in range(4):
                        xt = xpool8.tile([128, 2, 512], FP8, tag="xt8")
                        nc.sync.dma_start(
                            xt[:],
                            x8_d[s][:, m, :, t * 512:(t + 1) * 512],
                        )
                        xts.append(xt)
                else:
                    for dc in range(8):
                        xt = xpool.tile([128, 512], MMD, tag="xt")
                        nc.sync.dma_start(
                            xt[:],
                            xT[s][dc * 128:(dc + 1) * 128,
                                  t * 512:(t + 1) * 512],
                        )
                        xts.append(xt)
                x_pending[(s, t)] = xts

            def project_units(s, t):
                if t == 0:
                    load_weights(s)
                if s == "q" and t == 1:
                    nc.sync.dma_start(woT_sb[:], woT_d[:])
                prefetch_x(s, t)
                xts = x_pending.pop((s, t))
                for j in range(4):
                    pr = work.tile([128, 2, 512], F32, tag="wk")
                    if s == "v":
                        for dc in range(8):
                            # seq on partitions: lhsT = x chunk
                            nc.tensor.matmul(
                                pr[:, 0, :],
                                xts[dc][:, j * 128:(j + 1) * 128],
                                wsb["w1v"][:, dc, :],
                                start=(dc == 0), stop=False,
                            )
                            nc.tensor.matmul(
                                pr[:, 1, :],
                                xts[dc][:, j * 128:(j + 1) * 128],
                                wsb["w2v"][:, dc, :],
                                start=(dc == 0), stop=False,
                            )
                    else:
                        for m in range(4):
                            nc.tensor.matmul(
                                pr[:, 0, :],
                                wsb[f"w1{s}"][:, m, :,
                                              j * 128:(j + 1) * 128],
                                xts[m][:],
                                start=(m == 0), stop=(m == 3),
                                perf_mode=PERF.DoubleRow,
                            )
                            nc.tensor.matmul(
                                pr[:, 1, :],
                                wsb[f"w2{s}"][:, m, :,
                                              j * 128:(j + 1) * 128],
                                xts[m][:],
                                start=(m == 0), stop=(m == 3),
                                perf_mode=PERF.DoubleRow,
                            )
                    act = stage.tile([128, 512], F32, tag="act")
                    if s == "v":
                        # fold the biases into the accumulation (they vary
                        # along the free/channel dim)
                        nc.tensor.matmul(pr[:, 0, :], ones_r[:], bsb["b1v"][:],
                                         start=False, stop=True)
                        nc.tensor.matmul(pr[:, 1, :], ones_r[:], bsb["b2v"][:],
                                         start=False, stop=True)
                        nc.scalar.activation(act[:], pr[:, 0, :],
                                             ACTF.Tanh, scale=0.5)
                        u = stage.tile([128, 512], F32, tag="u")
                        # silu2(A) = (tanh(A/2) + 1) * A in one op
                        nc.vector.scalar_tensor_tensor(
                            u[:], act[:], 1.0, pr[:, 0, :],
                            op0=ALU.add, op1=ALU.mult,
                        )
                        src1 = pr[:, 1, :].rearrange("p (h d) -> p h d", h=8)
                        src2 = u[:].rearrange("p (h d) -> p h d", h=8)
                        nc.vector.tensor_tensor(
                            vb_t[t][:, j, :, 0:64], src1, src2, ALU.mult
                        )
                        if fp8_av:
                            with nc.allow_low_precision(reason="fp8 AV copy"):
                                nc.gpsimd.tensor_copy(
                                    v8_t[t][:, :, j, 0:64],
                                    vb_t[t][:, j, :, 0:64],
                                )
                    else:
                        bias1 = bsb[f"b1{s}"][:, j:j + 1]
                        bias2 = bsb[f"b2{s}"][:, j:j + 1]
                        # q/k PSUM carries 2^8 (fp8 weight pre-scale); fold
                        # the unscale into the tanh affine (b1h is unscaled)
                        nc.scalar.activation(
                            act[:], pr[:, 0, :], ACTF.Tanh,
                            scale=0.5 / 256.0,
                            bias=bsb[f"b1h{s}"][:, j:j + 1],
                        )
                        a_sb = stage.tile([128, 512], F32, tag="u")
                        nc.vector.tensor_scalar_add(a_sb[:], pr[:, 0, :],
                                                    bias1)
                        nc.vector.scalar_tensor_tensor(
                            act[:], act[:], 1.0, a_sb[:],
                            op0=ALU.add, op1=ALU.mult,
                        )
                        dst = (qt_t if s == "q" else kt_t)[t][:, j, :]
                        nc.vector.scalar_tensor_tensor(
                            dst, pr[:, 1, :], bias2, act[:],
                            op0=ALU.add, op1=ALU.mult,
                        )
                    yield

            def project(s, t):
                for _ in project_units(s, t):
                    pass

            # ---------------- phase B: one head pair of one query group -----
            ct_tiles = {}
            ctx_of = {}

            def attention_pair_units(qg, pj, mtiles):
                kcmax = kc_count(qg)
                use8 = fp8_av and qg >= 1
                if pj == 0:
                    ct_tiles[qg] = ctpool.tile([128, 4, 512], MMD, tag="ct",
                                               name=f"ct{qg}")
                ctx = [cxps.tile([128, 512], F32, tag="cx", name=f"ctx_{i}")
                       for i in range(2)]
                ctx_of[(qg, pj)] = ctx
                npair = (kcmax + 1) // 2
                for p in range(npair):
                    attn = (apool if use8 else apoolb).tile(
                        [128, 2, 2, 512], FP8 if use8 else MMD,
                        tag="at8" if use8 else "atb", name=f"at_{qg}_{pj}_{p}",
                    )
                    kcs = [kc for kc in (2 * p, 2 * p + 1) if kc < kcmax]
                    diag_adds = []
                    sc_of = {}
                    for kc in kcs:
                        c0 = col0(qg, kc)
                        tt, lkc = divmod(kc, 4)
                        diag = mask_mode == "causal" and kc >= 4 * qg
                        sc = work.tile([128, 2, 512], F32, tag="wk",
                                           name=f"sc_{qg}_{pj}_{kc}")
                        sc_of[kc] = sc
                        for par in range(2):
                            bp = par * 64
                            nc.tensor.matmul(
                                sc[:, par, c0:],
                                kt_t[tt][bp:bp + 64, pj,
                                         lkc * 128:(lkc + 1) * 128],
                                qt_t[qg][bp:bp + 64, pj, c0:],
                                start=True, stop=not diag,
                            )
                        if diag:
                            diag_adds.append((sc, c0))
                    # batched mask adds: mtri weights stay loaded across all
                    # diag subtiles of the pair (avoids kt<->mtri LDW thrash)
                    for sc, c0 in diag_adds:
                        for par in range(2):
                            nc.tensor.matmul(
                                sc[:, par, c0:c0 + 128],
                                ident_sb[:],
                                mtri_sb[:],
                                start=False, stop=True,
                            )
                    for kc in kcs:
                        c0 = col0(qg, kc)
                        sc = sc_of[kc]
                        with nc.allow_low_precision(reason="attn fp8"):
                            nc.scalar.activation(
                                attn[:, kc & 1, :, c0:], sc[:, :, c0:],
                                ACTF.Exp, scale=EXPS,
                            )
                        if mask_mode == "general":
                            for par in range(2):
                                nc.vector.tensor_tensor(
                                    attn[:, kc & 1, par, :],
                                    attn[:, kc & 1, par, :],
                                    mtiles[kc], ALU.mult,
                                )
                    # ---- AV ----
                    first = (p == 0)
                    last = (p == npair - 1)
                    anydiag = any(mask_mode == "causal" and kc >= 4 * qg
                                  for kc in kcs)
                    for par in range(2):
                        hl = 2 * pj + par
                        if use8 and not anydiag and len(kcs) == 2:
                            tt, l0 = divmod(2 * p, 4)
                            nc.tensor.matmul(
                                ctx[par][0:65, :],
                                v8_t[tt][:, hl, l0:l0 + 2, 0:65],
                                attn[:, :, par, :],
                                start=first, stop=last,
                                perf_mode=PERF.DoubleRow,
                                skip_group_check=True,
                            )
                        else:
                            for kc in kcs:
                                c0 = col0(qg, kc)
                                tt, lkc = divmod(kc, 4)
                                if use8:
                                    vt = v8_t[tt][:, hl, lkc, 0:65]
                                else:
                                    vt = vb_t[tt][:, lkc, hl, :]
                                nc.tensor.matmul(
                                    ctx[par][0:65, c0:],
                                    vt,
                                    attn[:, kc & 1, par, c0:],
                                    start=(first and kc == kcs[0]),
                                    stop=(last and kc == kcs[-1]),
                                    skip_group_check=True,
                                )
                    yield

            def normalize_unit(qg, pj):
                # ---- normalize both heads of the pair into ct_qg ----
                ct_qg = ct_tiles[qg]
                ctx = ctx_of.pop((qg, pj))
                for par in range(2):
                    nc.vector.tensor_copy(
                        den_sb[32 * par:32 * par + 1, :],
                        ctx[par][64:65, :],
                    )
                rec = smalls.tile([33, 512], F32, tag="rec")
                nc.vector.reciprocal_approx_fast(rec[:], den_sb[:])
                rec_b = smalls.tile([33, 512], MMD, tag="recb")
                nc.vector.tensor_copy(rec_b[:], rec[:])
                # broadcast each reciprocal row into the free upper rows
                # (64:128) of its own ctx PSUM bank
                for par in range(2):
                    nc.tensor.matmul(
                        ctx[par][64:128, :],
                        ones33[32 * par:32 * par + 1, 0:64],
                        rec_b[32 * par:32 * par + 1, :],
                        start=True, stop=True,
                        tile_position=(32 * par, 64),
                        skip_group_check=True,
                    )
                bc_sb = smalls.tile([128, 512], F32, tag="bcs")
                for par in range(2):
                    nc.vector.tensor_copy(
                        bc_sb[64 * par:64 * par + 64, :],
                        ctx[par][64:128, :],
                    )
                for par in range(2):
                    bp = par * 64
                    nc.vector.tensor_tensor(
                        ct_qg[bp:bp + 64, pj, :],
                        ctx[par][0:64, :],
                        bc_sb[64 * par:64 * par + 64, :],
                        ALU.mult,
                    )

            def attention_part(qg, pj, mtiles):
                for _ in attention_pair_units(qg, pj, mtiles):
                    pass
                normalize_unit(qg, pj)

            def attention_chain(qg, mtiles):
                for pj in range(4):
                    for _ in attention_pair_units(qg, pj, mtiles):
                        yield
                    normalize_unit(qg, pj)
                    yield

            def proj_stream(s, t, prefetches=()):
                for j, _ in enumerate(project_units(s, t)):
                    if j == 1:
                        for nx in prefetches:
                            prefetch_x(*nx)
                    yield

            def proj_chain(t):
                nxt = {"q": [("k", t)], "k": [("v", t)],
                       "v": [("q", t + 1)] if t < 3 else []}
                for s in "qkv":
                    yield from proj_stream(s, t, nxt[s])

            def weave(main_gen, side_gen, per_unit):
                """emit per_unit side units after each main unit (fractional
                credits), then drain whatever remains."""
                credit = 0.0
                for _ in main_gen:
                    credit += per_unit
                    while credit >= 1.0:
                        if next(side_gen, None) is None:
                            credit = 0.0
                            break
                        credit -= 1.0
                for _ in side_gen:
                    pass

            def attention_out_units(qg):
                ct_qg = ct_tiles[qg]
                for ns in range(4):
                    nt_i = qg * 4 + ns
                    nsl = slice(ns * 128, (ns + 1) * 128)
                    po = work.tile([128, 2, 512], F32, tag="wk",
                                   name=f"po_{qg}_{ns}")
                    for oh in range(2):
                        for j in range(4):
                            nc.tensor.matmul(
                                po[:, oh, :],
                                ct_qg[:, j, nsl],
                                woT_sb[:, j, oh * 512:(oh + 1) * 512],
                                start=(j == 0), stop=(j == 3),
                            )
                    ot = ostage.tile([128, 1024], F32, tag="ot")
                    nc.vector.tensor_copy(ot[:], po[:])
                    # gpsimd DMA queue: keeps the output stores off the sync
                    # queue that feeds the x prefetches
                    nc.gpsimd.dma_start(
                        pout_d[nt_i * 128:(nt_i + 1) * 128, :],
                        ot[:],
                    )
                    yield

            def attention_out(qg):
                for _ in attention_out_units(qg):
                    pass

            def general_mtiles(qg):
                if mask_mode != "general":
                    return None
                kcmax = kc_count(qg)
                mtiles = []
                mt_sb = mpool.tile([128, NT, 512], MMD, tag="mt")
                for kc in range(kcmax):
                    nc.sync.dma_start(
                        mt_sb[:, kc, :],
                        m01T_d[kc * 128:(kc + 1) * 128,
                               qg * 512:(qg + 1) * 512],
                    )
                    mtiles.append(mt_sb[:, kc, :])
                return mtiles

            # ---------------- interleaved schedule ----------------
            # Unit-level weave: attention(qg=t-1)'s kc-pair units are the main
            # stream; the three projection streams of slice t (plus, from t=2,
            # the output projection of group t-2) are the side stream.  The
            # fine interleave keeps proj matmuls between attention pairs so
            # the PE never outruns ACT's exp stream (which is slower per kc
            # pair than the PE work it gates), and the HAM clock stays warm.
            deferred_init()
            if mask_mode == "causal":
                import itertools

                for _ in proj_chain(0):
                    pass
                for t in range(1, 4):
                    qg = t - 1
                    main = attention_chain(qg, None)
                    side = proj_chain(t)
                    n_main = 4 * (((kc_count(qg) + 1) // 2) + 1)
                    n_side = 12
                    if qg == 2:
                        side = itertools.chain(side, attention_out_units(0))
                        n_side += 4
                    weave(main, side, n_side / n_main)
                main = attention_chain(3, None)
                side = itertools.chain(attention_out_units(1),
                                       attention_out_units(2))
                weave(main, side, 8 / 36)
                attention_out(3)
            else:
                for t in range(4):
                    for s in "qkv":
                        project(s, t)
                for qg in range(4):
                    mtiles = general_mtiles(qg)
                    for pj in range(4):
                        attention_part(qg, pj, mtiles)
                    attention_out(qg)
    nc.compile()
    return nc


def _host_prepare(inputs):
    """Split the full problem into 8 per-core input maps + host-side info."""
    q = np.asarray(inputs["query"], dtype=np.float32)
    k = np.asarray(inputs["key"], dtype=np.float32)
    v = np.asarray(inputs["value"], dtype=np.float32)
    mask = np.asarray(inputs["mask"])
    w = {n: np.asarray(inputs[n], dtype=np.float32)
         for n in ("wq1", "wq2", "wk1", "wk2", "wv1", "wv2", "wo")}
    bias = {n: np.asarray(inputs[n], dtype=np.float32)
            for n in ("bq1", "bq2", "bk1", "bk2", "bv1", "bv2", "bo")}

    m = mask.reshape(S, S)
    if np.array_equal(m != 0, np.tril(np.ones((S, S), bool))):
        mask_mode = "causal"
    elif np.all(m != 0):
        mask_mode = "full"
    else:
        mask_mode = "general"

    m01T = None
    if mask_mode == "general":
        m01T = np.ascontiguousarray((m != 0).T.astype(np.float32))

    scale = 1.0 / np.sqrt(DK).astype(np.float32)

    if MM_DTYPE == "bf16":
        import ml_dtypes

        mmd_np = ml_dtypes.bfloat16
    else:
        mmd_np = np.float32

    def cvt(a):
        return np.ascontiguousarray(a).astype(mmd_np)

    kk = np.arange(128)[:, None]
    qq = np.arange(128)[None, :]
    # scores carry 2^32 (each of qt/kt carries 2^16); the mask add must be in
    # the same scaled units (folded back out by the Exp scale argument)
    mtri = (kk > qq).astype(np.float32) * (NEG * 2.0 ** 32)
    ident = np.eye(128, dtype=np.float32)

    import ml_dtypes as mld

    def pack8(a):
        """[D, n] -> fp8 DoubleRow layout [128, 4, 2, n]."""
        a = np.asarray(a, np.float32).reshape(4, 2, 128, -1)
        a = np.clip(a, -240.0, 240.0).transpose(2, 0, 1, 3)
        return np.ascontiguousarray(a).astype(mld.float8_e4m3)

    WS = 256.0   # fp8 weight pre-scale (2^8)

    in_maps = []
    for c in range(NCORES):
        b, g = divmod(c, 2)
        sl = slice(g * GCH, (g + 1) * GCH)
        im = {
            "x8q": pack8(q[b].T),
            "x8k": pack8(k[b].T),
            "xvT": cvt(v[b].T),
            "w18_q": pack8(w["wq1"][sl].T * WS),
            # fold the 1/sqrt(dk) score scale into the non-silu Q branch,
            # and 0.5 everywhere (silu computed as A*(1+tanh(A/2)) = 2*silu)
            "w28_q": pack8(w["wq2"][sl].T * (scale * 0.5 * WS)),
            "w18_k": pack8(w["wk1"][sl].T * WS),
            "w28_k": pack8(w["wk2"][sl].T * (0.5 * WS)),
            "w1T_v": cvt(w["wv1"][sl].T),
            "w2T_v": cvt(w["wv2"][sl].T * 0.5),
            "b1_q": np.ascontiguousarray(
                (bias["bq1"][sl] * WS).reshape(4, 128).T),
            "b1h_q": np.ascontiguousarray(
                (bias["bq1"][sl] * 0.5).reshape(4, 128).T),
            "b2_q": np.ascontiguousarray(
                (bias["bq2"][sl] * (scale * 0.5 * WS)).reshape(4, 128).T),
            "b1_k": np.ascontiguousarray(
                (bias["bk1"][sl] * WS).reshape(4, 128).T),
            "b1h_k": np.ascontiguousarray(
                (bias["bk1"][sl] * 0.5).reshape(4, 128).T),
            "b2_k": np.ascontiguousarray(
                (bias["bk2"][sl] * (0.5 * WS)).reshape(4, 128).T),
            "b1_v": cvt(bias["bv1"][sl].reshape(1, GCH)),
            "b2_v": cvt((bias["bv2"][sl] * 0.5).reshape(1, GCH)),
            "woT": cvt(
                w["wo"][:, sl].T.reshape(4, 128, D).transpose(1, 0, 2)),
            "ident": cvt(ident),
        }
        if mask_mode == "causal":
            im["mtri"] = cvt(mtri)
        elif mask_mode == "general":
            im["m01T"] = cvt(m01T)
        in_maps.append(im)
    return mask_mode, in_maps, bias["bo"]


LAST_NC = None


def kernel(**inputs):
    global LAST_RESULT, LAST_NC
    mask_mode, in_maps, bo = _host_prepare(inputs)
    nc = build_program(mask_mode)
    LAST_NC = nc

    import concourse.bass_utils as bu

    if TRACE:
        import types

        try:
            from trn_agent_boot.trn_boot import _ntff_profile_via_ctypes

            hook = _ntff_profile_via_ctypes("/opt/axon/libaxon_pjrt.so")
            m = types.ModuleType("antenv.axon_hooks")
            m.get_axon_ntff_profile_hook = lambda: hook
            import antenv  # noqa: F401

            sys.modules["antenv.axon_hooks"] = m
            bu.upload_artifacts = lambda d: "local://skipped"
        except Exception as e:
            print("profiling hook install failed:", e)

    res = bu.run_bass_kernel_spmd(
        nc, in_maps, core_ids=list(range(NCORES)),
        trace=TRACE, trace_cores=TRACE_CORES,
    )
    LAST_RESULT = res

    out = np.empty((B, S, D), dtype=np.float32)
    for b in range(B):
        out[b] = (res.results[2 * b]["pout"] + res.results[2 * b + 1]["pout"]
                  + bo[None, :])
    return out

